# revision 6
# baseline (speedup 1.0000x reference)
"""Multi-head causal self-attention (B=2, S=2048, E=1024, H=16) on 8 TRN2 NeuronCores.

Sharding: tensor-parallel over heads (2 heads/core, both batches). Per core:
  - QKV projection for its 2 heads (q^T/k^T transposed layout, v natural;
    the v bias is deferred -- softmax rows sum to 1, so bv is a cheap
    per-partition DVE add after normalization)
  - causal attention computed transposed and WAVEFRONTED: token chunks are
    processed prefix-first (c=0..3); scores for 1024-wide column group g
    (cols [1024g, 1024g+1024)) are matmul'd + exp'd right after chunks
    2g/2g+1 land, so the ACT (exp) stream -- the binding engine at ~86us --
    flows continuously instead of bunching after the projection.
  - PV (flipped: lhsT = [v | ones], rhs = exp(scores^T)) runs per 512-wide
    q-tile as soon as its column group's exps exist; row 64 carries the
    softmax denominator; K=1 matmul broadcast + reciprocal + DVE multiply
    normalize into per-head attnT halves.
  - FOUR 256KB AllToAlls (one per batch-half): A2A(b, t) covers q cols
    [1024t, 1024t+1024) and fires the moment PV tiles 2t, 2t+1 finish --
    mid-scores for t=0 -- so collectives overlap compute instead of
    serializing at the end.
  - output projection pieces (one 128-token block per (b, t)) are woven
    into later ACT-bound stretches to keep the PE warm; the last piece
    (1,1) is the only work after the final collective.
Host side only reshapes/slices inputs and concatenates the 8 disjoint row
shards of the output.
"""

import numpy as np
import ml_dtypes

P = 128
B, S, E, H, D = 2, 2048, 1024, 16, 64
NCORES = 8
EB = E // P            # 8 e-blocks
BS = B * S             # 4096 flattened rows
SBB = S // P           # 16 s-blocks per batch
SB = BS // P           # 32 s-blocks global
HPC = H // NCORES      # 2 heads per core
QT = 512               # q-tile width for the PV phase
NQT = S // QT          # 4 q-tiles per batch

_bf16 = ml_dtypes.bfloat16
_cache = {}


def _build(no_cc=False):
    from contextlib import ExitStack

    import concourse.tile as tile
    from concourse import bacc, mybir

    bf16 = mybir.dt.bfloat16
    f32 = mybir.dt.float32

    nc = bacc.Bacc("TRN2", target_bir_lowering=False, debug=False,
                   num_devices=NCORES)

    # host-side layouts: x is pre-chunked so each 512-token load is one DMA
    # with 8KB-contiguous per-partition runs
    xT_d = nc.dram_tensor("xT", [B, NQT, P, EB, 512], bf16,
                          kind="ExternalInput")
    wqk_d = nc.dram_tensor("wqk", [P, EB, 2 * P], bf16, kind="ExternalInput")
    wv_d = nc.dram_tensor("wv", [P, EB, P], bf16, kind="ExternalInput")
    wo_d = nc.dram_tensor("wo", [P, EB, E], bf16, kind="ExternalInput")
    bqk_d = nc.dram_tensor("bqk", [P, 2], f32, kind="ExternalInput")
    bvc_d = nc.dram_tensor("bvc", [D, HPC], f32, kind="ExternalInput")
    bo_d = nc.dram_tensor("bo", [P, E], bf16, kind="ExternalInput")
    tri_d = nc.dram_tensor("tri", [P, P], bf16, kind="ExternalInput")
    # rank r owns token blocks {t*8 + r : t in 0,1} per batch; A2A (b, t)
    # moves q cols [1024t, 1024t+1024): dest rank j gets block t*8+j.
    out_d = nc.dram_tensor("out", [B, 2, P, E], f32, kind="ExternalOutput")
    a2a_in = [[nc.dram_tensor(f"a2a_in{b}{t}", [NCORES, P, P], bf16)
               for t in range(2)] for b in range(B)]
    a2a_out = [[nc.dram_tensor(f"a2a_out{b}{t}", [NCORES, P, P], bf16)
                for t in range(2)] for b in range(B)]

    with tile.TileContext(nc) as tc, ExitStack() as ctx:
        consts = ctx.enter_context(tc.tile_pool(name="consts", bufs=1))
        work = ctx.enter_context(tc.tile_pool(name="work", bufs=1))
        xpool = ctx.enter_context(tc.tile_pool(name="xstream", bufs=2))
        epool = ctx.enter_context(tc.tile_pool(name="expst", bufs=2))
        small = ctx.enter_context(tc.tile_pool(name="small", bufs=2))
        opool = ctx.enter_context(tc.tile_pool(name="osb", bufs=1))
        pbig = ctx.enter_context(tc.tile_pool(name="pbig", bufs=1, space="PSUM"))
        ppv = ctx.enter_context(tc.tile_pool(name="ppv", bufs=1, space="PSUM"))
        psm = ctx.enter_context(tc.tile_pool(name="psm", bufs=2, space="PSUM"))

        wqk = consts.tile([P, EB, 2 * P], bf16, tag="wqk")
        wv = consts.tile([P, EB, P], bf16, tag="wv")
        woh = [consts.tile([P, EB, 512], bf16, tag=f"wo{oh}",
                           name=f"wo{oh}") for oh in range(2)]
        bqk = consts.tile([P, 2], f32, tag="bqk")
        bo = consts.tile([P, E], bf16, tag="bo")
        tri = consts.tile([P, P], bf16, tag="tri")
        # all-ones column block: row 64 serves as the K=1 stationary
        # operand that broadcasts the denominator row
        onesc = consts.tile([P, D], bf16, tag="onesc")
        bvc = consts.tile([D, HPC], f32, tag="bvc")

        # first q/k matmul needs only wqk eb0 + the first x eb0 slice:
        # split the startup loads per-eb so PE starts ASAP
        for eb in range(EB):
            nc.sync.dma_start(wqk[:, eb, :], wqk_d[:, eb, :])
        nc.vector.memset(onesc[:], 1.0)

        qkT = [work.tile([P, 2, S], bf16, tag=f"qkT{b}", name=f"qkT{b}")
               for b in range(B)]
        vsb = [work.tile([P, SBB, HPC, 66], bf16, tag=f"vsb{b}", name=f"vsb{b}")
               for b in range(B)]
        # per-head attnT halves (both on partitions 0-63): keeps every DVE
        # normalize op partition-aligned; the bounce DMA shifts head 1 into
        # partitions 64-127 of the A2A payload
        attnT = [[work.tile([D, S], bf16, tag=f"attnT{h}",
                            name=f"attnT{b}{h}") for h in range(HPC)]
                 for b in range(B)]
        # exp(scores^T) tiles, allocated lazily per (b, h, kb); tag shared
        # across (b, h) with enough bufs for all in-flight users
        ets = {}

        def qkv_pieces(b):
            """QKV projection for batch b, chunks prefix-first; one psum
            group per piece (2 q/k + 4 v groups per 512-token chunk)."""
            nc.vector.memset(vsb[b][:], 1.0)
            for c in range(NQT):
                xc = xpool.tile([P, EB, 512], bf16, tag="xc", name="xc")
                if b == 0 and c == 0:
                    # eb-split so matmul eb=0 starts after ~1/8 of the bytes;
                    # remaining consts queue behind off the critical path
                    for eb in range(EB):
                        nc.sync.dma_start(xc[:, eb, :], xT_d[b, c, :, eb, :])
                        if eb == 0:
                            nc.sync.dma_start(bqk[:], bqk_d[:, :])
                    nc.sync.dma_start(wv[:], wv_d[:, :, :])
                    nc.sync.dma_start(bvc[:], bvc_d[:, :])
                    nc.sync.dma_start(tri[:], tri_d[:, :])
                else:
                    nc.sync.dma_start(xc[:], xT_d[b, c])
                for db in range(2):
                    ps = psm.tile([P, 512], f32, tag="mid", name="psqk")
                    for eb in range(EB):
                        nc.tensor.matmul(
                            ps[:],
                            lhsT=wqk[:, eb, db * P:(db + 1) * P],
                            rhs=xc[:, eb, :],
                            start=(eb == 0), stop=(eb == EB - 1),
                        )
                    nc.vector.tensor_scalar_add(
                        qkT[b][:, db, c * 512:(c + 1) * 512], ps[:],
                        bqk[:, db:db + 1])
                    yield
                for si in range(4):
                    sb = c * 4 + si
                    pv_ = psm.tile([P, P], f32, tag="mid", name="psv")
                    for eb in range(EB):
                        nc.tensor.matmul(
                            pv_[:], lhsT=xc[:, eb, si * P:(si + 1) * P],
                            rhs=wv[:, eb, :], start=(eb == 0),
                            stop=(eb == EB - 1))
                    # v bias deferred: attn = (exp@v)/denom + bv
                    nc.vector.tensor_copy(vsb[b][:, sb, 0, 0:64], pv_[:, 0:64])
                    nc.vector.tensor_copy(vsb[b][:, sb, 1, 0:64], pv_[:, 64:128])
                    yield

        def score_group(b, g):
            """scores^T + exp for column group g (cols [1024g, 1024g+1024))
            of batch b, both heads; one piece per k-block. Emitted after
            chunks 2g and 2g+1, so every operand is already written."""
            c0, c1 = 1024 * g, 1024 * g + 1024
            for kb in range(8 * g + 8):
                off = kb * P
                pss = []
                for h in range(HPC):
                    hs = slice(h * 64, (h + 1) * 64)
                    ps = pbig.tile([P, 1024], f32, tag=f"sc{h}", name=f"sc{h}")
                    lo = max(c0, off)
                    for m0 in range(c0, c1, 512):
                        s0 = max(m0, lo)
                        if s0 >= m0 + 512:
                            continue
                        nc.tensor.matmul(
                            ps[:, s0 - c0:m0 + 512 - c0],
                            lhsT=qkT[b][hs, 1, off:off + P],
                            rhs=qkT[b][hs, 0, s0:m0 + 512],
                            start=True, stop=True)
                    pss.append((ps, lo))
                for h in range(HPC):
                    ps, lo = pss[h]
                    key = (b, h, kb)
                    if key not in ets:
                        ets[key] = epool.tile(
                            [P, S - off], bf16, tag=f"e{kb}",
                            name=f"e{b}{h}{kb}", bufs=3)
                    et = ets[key]
                    nc.scalar.activation(
                        et[:, lo - off:c1 - off], ps[:, lo - c0:],
                        mybir.ActivationFunctionType.Exp)
                    if lo == off:
                        # diagonal block: zero the invalid (q < k) half
                        nc.vector.tensor_mul(et[:, 0:P], et[:, 0:P], tri[:])
                yield

        def pv_tile(b, qt):
            """Flipped PV for one 512-wide q-tile: per head, accumulate
            over k-blocks 0..4qt+3, then broadcast-normalize into attnT."""
            q0 = qt * QT
            pvs = [None, None]
            for h in range(HPC):
                pp = ppv.tile([65, QT], f32, tag=f"pv{h}", name=f"pv{h}")
                nkb = 4 * qt + 4
                for kb in range(nkb):
                    ecol = q0 - kb * P
                    poff = max(0, -ecol)
                    nc.tensor.matmul(
                        pp[:, poff:QT],
                        lhsT=vsb[b][:, kb, h, 0:65],
                        rhs=ets[(b, h, kb)][:, ecol + poff:ecol + QT],
                        start=(kb == 0), stop=(kb == nkb - 1))
                # fast-release: one DVE copy frees the PSUM slot
                pvs[h] = small.tile([65, QT], bf16, tag=f"pvs{h}",
                                    name=f"pvs{h}", bufs=1)
                nc.vector.tensor_copy(pvs[h][:], pp[:, :])
                yield
            for h in range(HPC):
                bc = psm.tile([D, QT], f32, tag="mid", name="bc")
                nc.tensor.matmul(bc[0:D, :],
                                 lhsT=onesc[64:65, 0:D],
                                 rhs=pvs[h][64:65, :],
                                 start=True, stop=True)
                bcs = small.tile([D, QT], f32, tag="bcs",
                                 name=f"bcs{h}", bufs=1)
                nc.vector.reciprocal_approx_fast(out=bcs[:], in_=bc[0:D, :])
                nc.vector.tensor_mul(attnT[b][h][0:D, q0:q0 + QT],
                                     pvs[h][0:D, :], bcs[0:D, :])
                nc.vector.tensor_scalar_add(
                    attnT[b][h][0:D, q0:q0 + QT],
                    attnT[b][h][0:D, q0:q0 + QT], bvc[0:D, h:h + 1])
            yield

        def bounce_and_a2a(b, t):
            """attnT cols [1024t, 1024t+1024) -> a2a_in payload -> AllToAll.
            On the otherwise-empty GpSimd SWDGE queue so the collective
            trigger right behind it fires immediately."""
            for h in range(HPC):
                nc.gpsimd.dma_start(
                    a2a_in[b][t].ap().rearrange(
                        "j p c -> p j c")[h * D:(h + 1) * D],
                    attnT[b][h][:, t * NCORES * P:(t + 1) * NCORES * P]
                    .rearrange("p (j c) -> p j c", c=P))
            if no_cc:
                for j in range(NCORES):
                    nc.sync.dma_start(a2a_out[b][t][j], a2a_in[b][t][j])
            else:
                nc.gpsimd.collective_compute(
                    "AllToAll", mybir.AluOpType.bypass,
                    replica_groups=[list(range(NCORES))],
                    ins=[a2a_in[b][t].ap()], outs=[a2a_out[b][t].ap()])

        def oproj(b, t):
            """Output projection of token block t*8 + rank of batch b.
            Gather on the GpSimd queue right after its own collective (in
            order, so no cross-queue stall); out DMA on the Vector queue."""
            atf = work.tile([P, NCORES, P], bf16, tag=f"atf{b}{t}",
                            name=f"atf{b}{t}")
            nc.gpsimd.dma_start(
                atf[:, :, :], a2a_out[b][t].ap().rearrange("j p c -> p j c"))
            ot = opool.tile([P, E], f32, tag="o", name="ot")
            for oh in range(2):
                po = psm.tile([P, 512], f32, tag="mid", name="pso")
                for j in range(NCORES):
                    nc.tensor.matmul(
                        po[:], lhsT=atf[:, j, :], rhs=woh[oh][:, j, :],
                        start=(j == 0), stop=(j == NCORES - 1))
                nc.vector.tensor_tensor(
                    out=ot[:, oh * 512:(oh + 1) * 512], in0=po[:],
                    in1=bo[:, oh * 512:(oh + 1) * 512],
                    op=mybir.AluOpType.add)
                yield
            nc.gpsimd.dma_start(out_d[b, t], ot[:])
            yield

        def drain(*gens):
            gens = list(gens)
            while gens:
                gens = [g for g in gens if next(g, StopIteration)
                        is not StopIteration]

        # ---- emission schedule ----
        # Per batch: chunks 0,1 -> [weave A: score g0 (ACT-heavy) + chunks
        # 2,3 + PV qt0/qt1 + A2A(b,0)] -> [weave B: score g1 + next batch's
        # chunks 0,1 + PV qt2/qt3 + one landed oproj piece + A2A(b,1)].
        # oproj placement: (0,0) in b0 weave B, (0,1) in b1 weave B, (1,0)
        # in the tail covering A2A(1,1), (1,1) last.
        qg = {b: qkv_pieces(b) for b in range(B)}

        def take(g, n):
            for _ in range(n):
                next(g, None)

        # batch 0 chunks 0,1
        take(qg[0], 12)
        nc.sync.dma_start(bo[:, :], bo_d[:, :])
        for oh in range(2):
            nc.sync.dma_start(woh[oh][:], wo_d[:, :, oh * 512:(oh + 1) * 512])

        for b in range(B):
            # weave A: score group 0 (8 pieces) with chunks 2,3 (12 qkv
            # pieces) and PV qt0 (after kb3) / qt1 (after kb7)
            sg = score_group(b, g=0)
            for kb in range(8):
                next(sg, None)
                if kb < 6:
                    take(qg[b], 2)
                if kb == 3:
                    drain(pv_tile(b, 0))
                if kb == 7:
                    drain(pv_tile(b, 1))
            bounce_and_a2a(b, 0)
            # weave B: score group 1 (16 pieces) with next batch's chunks
            # 0,1, PV qt2 (after kb11) / qt3 (after kb15), one oproj
            sg = score_group(b, g=1)
            nb = b + 1
            for kb in range(16):
                next(sg, None)
                if nb < B and kb < 12:
                    take(qg[nb], 1)
                if kb == 8:
                    # A2A(b-.., ) long landed: b0 gets (0,0), b1 gets (0,1)
                    drain(oproj(0, b))
                if kb == 11:
                    drain(pv_tile(b, 2))
                if kb == 15:
                    drain(pv_tile(b, 3))
            bounce_and_a2a(b, 1)

        # tail: oproj(1,0) (its A2A fired before weave B, long done)
        # overlaps the final collective; oproj(1,1) is the only work after
        drain(oproj(1, 0))
        drain(oproj(1, 1))

    nc.compile()
    return nc


def _in_maps(x, W_qkv, b_qkv, W_o, b_o):
    xT = np.ascontiguousarray(
        x.reshape(B, NQT, 512, EB, P).transpose(0, 1, 4, 3, 2)).astype(_bf16)
    wo = np.ascontiguousarray(
        W_o.reshape(EB, P, E).transpose(1, 0, 2)).astype(_bf16)
    bo = np.broadcast_to(np.asarray(b_o).reshape(1, E), (P, E)).astype(_bf16)
    tri = np.triu(np.ones((P, P), np.float32)).astype(_bf16)
    maps = []
    for c in range(NCORES):
        o = c * HPC * D
        q_sl = slice(o, o + HPC * D)
        k_sl = slice(E + o, E + o + HPC * D)
        v_sl = slice(2 * E + o, 2 * E + o + HPC * D)
        wqk = np.concatenate(
            [W_qkv[:, q_sl] * 0.125, W_qkv[:, k_sl]], axis=1)
        maps.append({
            "xT": xT,
            "wqk": np.ascontiguousarray(
                wqk.reshape(EB, P, 2 * P).transpose(1, 0, 2)).astype(_bf16),
            "wv": np.ascontiguousarray(
                W_qkv[:, v_sl].reshape(EB, P, P).transpose(1, 0, 2)).astype(_bf16),
            "wo": wo,
            "bqk": np.stack([b_qkv[q_sl] * 0.125,
                             b_qkv[k_sl]], axis=1).astype(np.float32),
            "bvc": np.ascontiguousarray(
                b_qkv[v_sl].reshape(HPC, D).T).astype(np.float32),
            "bo": bo,
            "tri": tri,
        })
    return maps


def kernel(x, W_qkv, b_qkv, W_o, b_o, mask):
    from concourse.bass_utils import run_bass_kernel_spmd

    if "nc" not in _cache:
        _cache["nc"] = _build()
    nc = _cache["nc"]
    maps = _in_maps(np.asarray(x, np.float32), np.asarray(W_qkv, np.float32),
                    np.asarray(b_qkv, np.float32), np.asarray(W_o, np.float32),
                    np.asarray(b_o, np.float32))
    res = run_bass_kernel_spmd(nc, maps, list(range(NCORES)))
    # rank r's out[b, t] is token block t*8 + r of batch b
    full = np.empty((B, SBB, P, E), np.float32)
    for r in range(NCORES):
        blk = res.results[r]["out"]          # [B, 2, P, E]
        for t in range(2):
            full[:, t * NCORES + r] = blk[:, t]
    return full.reshape(B, S, E).astype(np.float32)


# revision 12
# speedup vs baseline: 1.0237x; 1.0237x over previous
"""Multi-head causal self-attention (B=2, S=2048, E=1024, H=16) on 8 TRN2 NeuronCores.

Sharding: tensor-parallel over heads (2 heads/core, both batches). Per core:
  - QKV projection for its 2 heads (q^T/k^T transposed layout, v natural;
    the v bias is deferred -- softmax rows sum to 1, so bv is a cheap
    per-partition DVE add after normalization)
  - causal attention computed transposed and WAVEFRONTED: token chunks are
    processed prefix-first (c=0..3); scores for 1024-wide column group g
    (cols [1024g, 1024g+1024)) are matmul'd + exp'd right after chunks
    2g/2g+1 land, so the ACT (exp) stream -- the binding engine at ~86us --
    flows continuously instead of bunching after the projection.
  - PV (flipped: lhsT = [v | ones], rhs = exp(scores^T)) runs per 512-wide
    q-tile as soon as its column group's exps exist; row 64 carries the
    softmax denominator; K=1 matmul broadcast + reciprocal + DVE multiply
    normalize into per-head attnT halves.
  - FOUR 256KB AllToAlls (one per batch-half): A2A(b, t) covers q cols
    [1024t, 1024t+1024) and fires the moment PV tiles 2t, 2t+1 finish --
    mid-scores for t=0 -- so collectives overlap compute instead of
    serializing at the end.
  - output projection pieces (one 128-token block per (b, t)) are woven
    into later ACT-bound stretches to keep the PE warm; the last piece
    (1,1) is the only work after the final collective.
Host side only reshapes/slices inputs and concatenates the 8 disjoint row
shards of the output.
"""

import numpy as np
import ml_dtypes

P = 128
B, S, E, H, D = 2, 2048, 1024, 16, 64
NCORES = 8
EB = E // P            # 8 e-blocks
BS = B * S             # 4096 flattened rows
SBB = S // P           # 16 s-blocks per batch
SB = BS // P           # 32 s-blocks global
HPC = H // NCORES      # 2 heads per core
QT = 512               # q-tile width for the PV phase
NQT = S // QT          # 4 q-tiles per batch

_bf16 = ml_dtypes.bfloat16
_cache = {}


def _build(no_cc=False):
    from contextlib import ExitStack

    import concourse.tile as tile
    from concourse import bacc, mybir

    bf16 = mybir.dt.bfloat16
    f32 = mybir.dt.float32

    nc = bacc.Bacc("TRN2", target_bir_lowering=False, debug=False,
                   num_devices=NCORES)

    # host-side layouts: x is pre-chunked so each 512-token load is one DMA
    # with 8KB-contiguous per-partition runs
    xT_d = nc.dram_tensor("xT", [B, NQT, P, EB, 512], bf16,
                          kind="ExternalInput")
    wqk_d = nc.dram_tensor("wqk", [P, EB, 2 * P], bf16, kind="ExternalInput")
    wv_d = nc.dram_tensor("wv", [P, EB, P], bf16, kind="ExternalInput")
    wo_d = nc.dram_tensor("wo", [P, EB, E], bf16, kind="ExternalInput")
    bqk_d = nc.dram_tensor("bqk", [P, 2], f32, kind="ExternalInput")
    bvc_d = nc.dram_tensor("bvc", [D, HPC], f32, kind="ExternalInput")
    bo_d = nc.dram_tensor("bo", [P, E], bf16, kind="ExternalInput")
    tri_d = nc.dram_tensor("tri", [P, P], bf16, kind="ExternalInput")
    # rank r owns token blocks {t*8 + r : t in 0,1} per batch; one 512KB
    # AllToAll per batch (collectives have a ~13us fixed cost, so fewer,
    # larger ops win); dest rank j gets blocks {j, j+8}.
    out_d = nc.dram_tensor("out", [B, 2, P, E], bf16, kind="ExternalOutput")
    a2a_in = [nc.dram_tensor(f"a2a_in{b}", [NCORES, P, 2 * P], bf16)
              for b in range(B)]
    a2a_out = [nc.dram_tensor(f"a2a_out{b}", [NCORES, P, 2 * P], bf16)
               for b in range(B)]

    with tile.TileContext(nc) as tc, ExitStack() as ctx:
        consts = ctx.enter_context(tc.tile_pool(name="consts", bufs=1))
        work = ctx.enter_context(tc.tile_pool(name="work", bufs=1))
        xpool = ctx.enter_context(tc.tile_pool(name="xstream", bufs=3))
        epool = ctx.enter_context(tc.tile_pool(name="expst", bufs=2))
        small = ctx.enter_context(tc.tile_pool(name="small", bufs=2))
        opool = ctx.enter_context(tc.tile_pool(name="osb", bufs=1))
        pbig = ctx.enter_context(tc.tile_pool(name="pbig", bufs=1, space="PSUM"))
        ppv = ctx.enter_context(tc.tile_pool(name="ppv", bufs=1, space="PSUM"))
        psm = ctx.enter_context(tc.tile_pool(name="psm", bufs=2, space="PSUM"))

        wqk = consts.tile([P, EB, 2 * P], bf16, tag="wqk")
        wv = consts.tile([P, EB, P], bf16, tag="wv")
        woh = [consts.tile([P, EB, 512], bf16, tag=f"wo{oh}",
                           name=f"wo{oh}") for oh in range(2)]
        bqk = consts.tile([P, 2], f32, tag="bqk")
        bo = consts.tile([P, E], bf16, tag="bo")
        tri = consts.tile([P, P], bf16, tag="tri")
        # all-ones column block: row 64 serves as the K=1 stationary
        # operand that broadcasts the denominator row
        onesc = consts.tile([P, D], bf16, tag="onesc")
        bvc = consts.tile([D, HPC], f32, tag="bvc")

        nc.vector.memset(onesc[:], 1.0)

        qkT = [work.tile([P, 2, S], bf16, tag=f"qkT{b}", name=f"qkT{b}")
               for b in range(B)]
        vsb = [work.tile([P, SBB, HPC, 66], bf16, tag=f"vsb{b}", name=f"vsb{b}")
               for b in range(B)]
        # per-head attnT halves (both on partitions 0-63): keeps every DVE
        # normalize op partition-aligned; the bounce DMA shifts head 1 into
        # partitions 64-127 of the A2A payload
        attnT = [[work.tile([D, S], bf16, tag=f"attnT{h}",
                            name=f"attnT{b}{h}") for h in range(HPC)]
                 for b in range(B)]
        # exp(scores^T) tiles, allocated lazily per (b, h, kb); tag shared
        # across (b, h) with enough bufs for all in-flight users
        ets = {}

        def qkv_pieces(b):
            """QKV projection for batch b, chunks prefix-first; one psum
            group per piece (2 q/k + 4 v groups per 512-token chunk)."""
            nc.vector.memset(vsb[b][:], 1.0)
            for c in range(NQT):
                xc = xpool.tile([P, EB, 512], bf16, tag="xc", name="xc")
                if b == 0 and c == 0:
                    # interleave wqk/x per-eb so matmul eb=0 starts after
                    # ~1/8 of the startup bytes; remaining consts queue
                    # behind off the critical path
                    for eb in range(EB):
                        nc.sync.dma_start(wqk[:, eb, :], wqk_d[:, eb, :])
                        nc.sync.dma_start(xc[:, eb, :], xT_d[b, c, :, eb, :])
                        if eb == 0:
                            nc.sync.dma_start(bqk[:], bqk_d[:, :])
                    nc.sync.dma_start(wv[:], wv_d[:, :, :])
                    nc.sync.dma_start(bvc[:], bvc_d[:, :])
                    nc.sync.dma_start(tri[:], tri_d[:, :])
                else:
                    nc.sync.dma_start(xc[:], xT_d[b, c])
                for db in range(2):
                    ps = psm.tile([P, 512], f32, tag="mid", name="psqk")
                    for eb in range(EB):
                        nc.tensor.matmul(
                            ps[:],
                            lhsT=wqk[:, eb, db * P:(db + 1) * P],
                            rhs=xc[:, eb, :],
                            start=(eb == 0), stop=(eb == EB - 1),
                        )
                    nc.vector.tensor_scalar_add(
                        qkT[b][:, db, c * 512:(c + 1) * 512], ps[:],
                        bqk[:, db:db + 1])
                    yield
                for si in range(4):
                    sb = c * 4 + si
                    pv_ = psm.tile([P, P], f32, tag="mid", name="psv")
                    for eb in range(EB):
                        nc.tensor.matmul(
                            pv_[:], lhsT=xc[:, eb, si * P:(si + 1) * P],
                            rhs=wv[:, eb, :], start=(eb == 0),
                            stop=(eb == EB - 1))
                    # v bias deferred: attn = (exp@v)/denom + bv
                    nc.vector.tensor_copy(vsb[b][:, sb, 0, 0:64], pv_[:, 0:64])
                    nc.vector.tensor_copy(vsb[b][:, sb, 1, 0:64], pv_[:, 64:128])
                    yield

        def score_group(b, g):
            """scores^T + exp for column group g (cols [1024g, 1024g+1024))
            of batch b, both heads; one piece per k-block. Emitted after
            chunks 2g and 2g+1, so every operand is already written."""
            c0, c1 = 1024 * g, 1024 * g + 1024
            for kb in range(8 * g + 8):
                off = kb * P
                pss = []
                for h in range(HPC):
                    hs = slice(h * 64, (h + 1) * 64)
                    ps = pbig.tile([P, 1024], f32, tag=f"sc{h}", name=f"sc{h}")
                    lo = max(c0, off)
                    for m0 in range(c0, c1, 512):
                        s0 = max(m0, lo)
                        if s0 >= m0 + 512:
                            continue
                        nc.tensor.matmul(
                            ps[:, s0 - c0:m0 + 512 - c0],
                            lhsT=qkT[b][hs, 1, off:off + P],
                            rhs=qkT[b][hs, 0, s0:m0 + 512],
                            start=True, stop=True)
                    pss.append((ps, lo))
                for h in range(HPC):
                    ps, lo = pss[h]
                    key = (b, h, kb)
                    if key not in ets:
                        ets[key] = epool.tile(
                            [P, S - off], bf16, tag=f"e{kb}",
                            name=f"e{b}{h}{kb}", bufs=3)
                    et = ets[key]
                    nc.scalar.activation(
                        et[:, lo - off:c1 - off], ps[:, lo - c0:],
                        mybir.ActivationFunctionType.Exp)
                    if lo == off:
                        # diagonal block: zero the invalid (q < k) half
                        nc.vector.tensor_mul(et[:, 0:P], et[:, 0:P], tri[:])
                yield

        def pv_tile(b, qt):
            """Flipped PV for one 512-wide q-tile: per head, accumulate
            over k-blocks 0..4qt+3, then broadcast-normalize into attnT."""
            q0 = qt * QT
            pvs = [None, None]
            for h in range(HPC):
                pp = ppv.tile([65, QT], f32, tag=f"pv{h}", name=f"pv{h}")
                nkb = 4 * qt + 4
                for kb in range(nkb):
                    ecol = q0 - kb * P
                    poff = max(0, -ecol)
                    nc.tensor.matmul(
                        pp[:, poff:QT],
                        lhsT=vsb[b][:, kb, h, 0:65],
                        rhs=ets[(b, h, kb)][:, ecol + poff:ecol + QT],
                        start=(kb == 0), stop=(kb == nkb - 1))
                # fast-release: one DVE copy frees the PSUM slot
                pvs[h] = small.tile([65, QT], bf16, tag=f"pvs{h}",
                                    name=f"pvs{h}", bufs=1)
                nc.vector.tensor_copy(pvs[h][:], pp[:, :])
                yield
            for h in range(HPC):
                bc = psm.tile([D, QT], f32, tag="mid", name="bc")
                nc.tensor.matmul(bc[0:D, :],
                                 lhsT=onesc[64:65, 0:D],
                                 rhs=pvs[h][64:65, :],
                                 start=True, stop=True)
                bcs = small.tile([D, QT], f32, tag="bcs",
                                 name=f"bcs{h}", bufs=1)
                nc.vector.reciprocal_approx_fast(out=bcs[:], in_=bc[0:D, :])
                nc.vector.tensor_mul(attnT[b][h][0:D, q0:q0 + QT],
                                     pvs[h][0:D, :], bcs[0:D, :])
                nc.vector.tensor_scalar_add(
                    attnT[b][h][0:D, q0:q0 + QT],
                    attnT[b][h][0:D, q0:q0 + QT], bvc[0:D, h:h + 1])
            yield

        atfs = [work.tile([P, NCORES, 2 * P], bf16, tag=f"atf{b}",
                          name=f"atf{b}") for b in range(B)]

        def bounce_and_a2a(b):
            """attnT -> a2a_in payload -> one 512KB AllToAll for batch b,
            then the gather back into atf -- all on the GpSimd SWDGE queue
            (in order, so the gather naturally waits its own collective
            without stalling any compute queue)."""
            for t in range(2):
                for h in range(HPC):
                    nc.gpsimd.dma_start(
                        a2a_in[b].ap().rearrange(
                            "j p (t c) -> p j t c",
                            t=2)[h * D:(h + 1) * D, :, t, :],
                        attnT[b][h][:, t * NCORES * P:(t + 1) * NCORES * P]
                        .rearrange("p (j c) -> p j c", c=P))
            if no_cc:
                for j in range(NCORES):
                    nc.sync.dma_start(a2a_out[b][j], a2a_in[b][j])
            else:
                nc.gpsimd.collective_compute(
                    "AllToAll", mybir.AluOpType.bypass,
                    replica_groups=[list(range(NCORES))],
                    ins=[a2a_in[b].ap()], outs=[a2a_out[b].ap()])
            nc.gpsimd.dma_start(
                atfs[b][:, :, :], a2a_out[b].ap().rearrange("j p c -> p j c"))

        def oproj(b, t):
            """Output projection of token block t*8 + rank of batch b."""
            ot = opool.tile([P, E], bf16, tag="o", name="ot")
            for oh in range(2):
                po = psm.tile([P, 512], f32, tag="mid", name="pso")
                for j in range(NCORES):
                    nc.tensor.matmul(
                        po[:], lhsT=atfs[b][:, j, t * P:(t + 1) * P],
                        rhs=woh[oh][:, j, :],
                        start=(j == 0), stop=(j == NCORES - 1))
                nc.vector.tensor_tensor(
                    out=ot[:, oh * 512:(oh + 1) * 512], in0=po[:],
                    in1=bo[:, oh * 512:(oh + 1) * 512],
                    op=mybir.AluOpType.add)
                yield
            nc.gpsimd.dma_start(out_d[b, t], ot[:])
            yield

        def drain(*gens):
            gens = list(gens)
            while gens:
                gens = [g for g in gens if next(g, StopIteration)
                        is not StopIteration]

        # ---- emission schedule ----
        # Deep batch interleave so batch-0's wide exps (group g1) run
        # before batch-1's projection tail, keeping the late kernel from
        # going ACT-bound:
        #   b0c0,c1 | wA0: b0.g0 x {b0c2,c3; pv0.qt0} |
        #   wB0: b0.g1 x {b1c0,c1; pv0.qt1; pv0.qt2} |
        #   wA1: b1.g0 x {b1c2,c3; pv0.qt3 -> A2A(0); pv1.qt0; pv1.qt1} |
        #   wB1: b1.g1 x {oproj(0,*); pv1.qt2; pv1.qt3} | A2A(1) |
        #   tail: oproj(1,0) || A2A(1), oproj(1,1)
        qg = {b: qkv_pieces(b) for b in range(B)}

        def take(g, n):
            for _ in range(n):
                next(g, None)

        take(qg[0], 12)         # b0 chunks 0,1
        nc.sync.dma_start(bo[:, :], bo_d[:, :])
        for oh in range(2):
            nc.sync.dma_start(woh[oh][:], wo_d[:, :, oh * 512:(oh + 1) * 512])

        sg = score_group(0, g=0)
        for kb in range(8):     # wA0
            next(sg, None)
            if kb < 6:
                take(qg[0], 2)  # b0 chunks 2,3
            if kb == 3:
                drain(pv_tile(0, 0))
        sg = score_group(0, g=1)
        for kb in range(16):    # wB0
            next(sg, None)
            if kb < 12:
                take(qg[1], 1)  # b1 chunks 0,1
            if kb == 3:
                drain(pv_tile(0, 1))
            if kb == 11:
                drain(pv_tile(0, 2))
        sg = score_group(1, g=0)
        for kb in range(8):     # wA1
            next(sg, None)
            if kb < 6:
                take(qg[1], 2)  # b1 chunks 2,3
            if kb == 1:
                drain(pv_tile(0, 3))
                bounce_and_a2a(0)
            if kb == 3:
                drain(pv_tile(1, 0))
            if kb == 7:
                drain(pv_tile(1, 1))
        sg = score_group(1, g=1)
        for kb in range(16):    # wB1
            next(sg, None)
            if kb == 4:
                drain(oproj(0, 0))
            if kb == 8:
                drain(oproj(0, 1))
            if kb == 11:
                drain(pv_tile(1, 2))
            if kb == 15:
                drain(pv_tile(1, 3))
        bounce_and_a2a(1)

        # tail: oproj(1,0) fills the A2A(1) window once the gather lands;
        # oproj(1,1) is the only work after
        drain(oproj(1, 0))
        drain(oproj(1, 1))

    nc.compile()
    return nc


def _in_maps(x, W_qkv, b_qkv, W_o, b_o):
    xT = np.ascontiguousarray(
        x.reshape(B, NQT, 512, EB, P).transpose(0, 1, 4, 3, 2)).astype(_bf16)
    wo = np.ascontiguousarray(
        W_o.reshape(EB, P, E).transpose(1, 0, 2)).astype(_bf16)
    bo = np.broadcast_to(np.asarray(b_o).reshape(1, E), (P, E)).astype(_bf16)
    tri = np.triu(np.ones((P, P), np.float32)).astype(_bf16)
    maps = []
    for c in range(NCORES):
        o = c * HPC * D
        q_sl = slice(o, o + HPC * D)
        k_sl = slice(E + o, E + o + HPC * D)
        v_sl = slice(2 * E + o, 2 * E + o + HPC * D)
        wqk = np.concatenate(
            [W_qkv[:, q_sl] * 0.125, W_qkv[:, k_sl]], axis=1)
        maps.append({
            "xT": xT,
            "wqk": np.ascontiguousarray(
                wqk.reshape(EB, P, 2 * P).transpose(1, 0, 2)).astype(_bf16),
            "wv": np.ascontiguousarray(
                W_qkv[:, v_sl].reshape(EB, P, P).transpose(1, 0, 2)).astype(_bf16),
            "wo": wo,
            "bqk": np.stack([b_qkv[q_sl] * 0.125,
                             b_qkv[k_sl]], axis=1).astype(np.float32),
            "bvc": np.ascontiguousarray(
                b_qkv[v_sl].reshape(HPC, D).T).astype(np.float32),
            "bo": bo,
            "tri": tri,
        })
    return maps


def kernel(x, W_qkv, b_qkv, W_o, b_o, mask):
    from concourse.bass_utils import run_bass_kernel_spmd

    if "nc" not in _cache:
        _cache["nc"] = _build()
    nc = _cache["nc"]
    maps = _in_maps(np.asarray(x, np.float32), np.asarray(W_qkv, np.float32),
                    np.asarray(b_qkv, np.float32), np.asarray(W_o, np.float32),
                    np.asarray(b_o, np.float32))
    res = run_bass_kernel_spmd(nc, maps, list(range(NCORES)))
    # rank r's out[b, t] is token block t*8 + r of batch b
    full = np.empty((B, SBB, P, E), np.float32)
    for r in range(NCORES):
        blk = res.results[r]["out"]          # [B, 2, P, E]
        for t in range(2):
            full[:, t * NCORES + r] = blk[:, t]
    return full.reshape(B, S, E).astype(np.float32)


# revision 20
# speedup vs baseline: 1.0243x; 1.0006x over previous
"""Multi-head causal self-attention (B=2, S=2048, E=1024, H=16) on 8 TRN2 NeuronCores.

Sharding: tensor-parallel over heads (2 heads/core, both batches). Per core:
  - QKV projection for its 2 heads (q^T/k^T transposed layout, v natural;
    the v bias is deferred -- softmax rows sum to 1, so bv is a cheap
    per-partition DVE add after normalization instead of 64 K=1 matmuls)
  - causal flash-style attention, scores computed transposed (k on
    partitions) and exp'd on ACT in 1024-wide PSUM tiles; tri-mask on DVE
  - PV computed FLIPPED (lhsT = [v | ones], rhs = exp(scores^T)): each
    matmul streams 512 q-columns (matmul-bound, no LDWEIGHTS stalls) and
    the result lands pre-transposed [d, q] with the softmax denominator in
    row 64; a K=1 matmul broadcasts the denominator row across 64
    partitions, reciprocal_approx_fast inverts the broadcast, one DVE
    multiply normalizes into per-head attnT halves (all partition-aligned)
  - one 512KB AllToAll per batch, triggered from the otherwise-empty
    GpSimd queue the moment that batch's attnT completes (collectives
    serialize on TOPSP, so fewer, larger A2As beat four half-sized ones),
    then four output projections, each overlapping the next collective.
Emission is phase-sequential (in-order engine queues): qkv0+scores0
(reversed, paced), qkv1+scores1 (middle-out: high k-blocks paced with the
suffix-first chunks, low half forward after qkv1 so pv1 unlocks
incrementally), pv0, pv1, oproj x4. All 16 exp tiles per (batch, head) keep
4 slot buffers so batch-1 exps never wait on batch-0's PV to release them.
Host side only reshapes/slices inputs and concatenates the 8 disjoint row
shards of the output.
"""

import numpy as np
import ml_dtypes

P = 128
B, S, E, H, D = 2, 2048, 1024, 16, 64
NCORES = 8
EB = E // P            # 8 e-blocks
BS = B * S             # 4096 flattened rows
SBB = S // P           # 16 s-blocks per batch
SB = BS // P           # 32 s-blocks global
HPC = H // NCORES      # 2 heads per core
CH = BS // NCORES      # 512 rows owned per core
QT = 512               # q-tile width for the PV phase
NQT = S // QT          # 4 q-tiles per batch

_bf16 = ml_dtypes.bfloat16
_cache = {}


def _build(no_cc=False):
    from contextlib import ExitStack

    import concourse.tile as tile
    from concourse import bacc, mybir

    bf16 = mybir.dt.bfloat16
    f32 = mybir.dt.float32

    nc = bacc.Bacc("TRN2", target_bir_lowering=False, debug=False,
                   num_devices=NCORES)

    # host-side layouts are [partition, eblock, col] so each load is one DMA
    xT_d = nc.dram_tensor("xT", [P, EB, BS], bf16, kind="ExternalInput")
    wqk_d = nc.dram_tensor("wqk", [P, EB, 2 * P], bf16, kind="ExternalInput")
    wv_d = nc.dram_tensor("wv", [P, EB, P], bf16, kind="ExternalInput")
    wo_d = nc.dram_tensor("wo", [P, EB, E], bf16, kind="ExternalInput")
    bqk_d = nc.dram_tensor("bqk", [P, 2], f32, kind="ExternalInput")
    bvc_d = nc.dram_tensor("bvc", [D, HPC], f32, kind="ExternalInput")
    bo_d = nc.dram_tensor("bo", [P, E], bf16, kind="ExternalInput")
    tri_d = nc.dram_tensor("tri", [P, P], bf16, kind="ExternalInput")
    # rank r owns interleaved token blocks {r, r+8, r+16, r+24}: one AllToAll
    # per batch. out row-block st <-> global block st*8 + rank.
    out_d = nc.dram_tensor("out", [4, P, E], bf16, kind="ExternalOutput")
    a2a_in = [nc.dram_tensor(f"a2a_in{b}", [NCORES, P, 2 * P], bf16)
              for b in range(B)]
    a2a_out = [nc.dram_tensor(f"a2a_out{b}", [NCORES, P, 2 * P], bf16)
               for b in range(B)]

    with tile.TileContext(nc) as tc, ExitStack() as ctx:
        consts = ctx.enter_context(tc.tile_pool(name="consts", bufs=1))
        work = ctx.enter_context(tc.tile_pool(name="work", bufs=1))
        xpool = ctx.enter_context(tc.tile_pool(name="xstream", bufs=2))
        epool = ctx.enter_context(tc.tile_pool(name="expst", bufs=2))
        small = ctx.enter_context(tc.tile_pool(name="small", bufs=2))
        opool = ctx.enter_context(tc.tile_pool(name="osb", bufs=1))
        pbig = ctx.enter_context(tc.tile_pool(name="pbig", bufs=2, space="PSUM"))
        ppv = ctx.enter_context(tc.tile_pool(name="ppv", bufs=1, space="PSUM"))
        psm = ctx.enter_context(tc.tile_pool(name="psm", bufs=2, space="PSUM"))

        wqk = consts.tile([P, EB, 2 * P], bf16, tag="wqk")
        wv = consts.tile([P, EB, P], bf16, tag="wv")
        bqk = consts.tile([P, 2], f32, tag="bqk")
        bo = consts.tile([P, E], bf16, tag="bo")
        tri = consts.tile([P, P], bf16, tag="tri")
        ones1 = consts.tile([1, P], bf16, tag="ones1")
        # all-ones column block: row 64 serves as the K=1 stationary
        # operand that broadcasts the denominator row (also at partition 64)
        onesc = consts.tile([P, D], bf16, tag="onesc")
        bvc = consts.tile([D, HPC], f32, tag="bvc")

        nc.vector.memset(ones1[:1, :], 1.0)
        nc.vector.memset(onesc[:], 1.0)

        qkT = [work.tile([P, 2, S], bf16, tag=f"qkT{b}", name=f"qkT{b}")
               for b in range(B)]
        vsb = [work.tile([P, SBB, HPC, 66], bf16, tag=f"vsb{b}", name=f"vsb{b}")
               for b in range(B)]
        # per-head attnT halves (both on partitions 0-63): keeps every DVE
        # normalize op partition-aligned; the bounce DMA does the shift of
        # head 1 into partitions 64-127 of the A2A payload
        attnT = [[work.tile([D, S], bf16, tag=f"attnT{h}",
                            name=f"attnT{b}{h}") for h in range(HPC)]
                 for b in range(B)]

        def qkv_pieces(b):
            """QKV projection for batch b, one 512-token chunk per piece.

            Chunks are emitted suffix-first: causal score block kb only needs
            token columns >= kb*128, so late chunks unblock the small k-blocks
            early and ACT (exp) can start before the whole projection is done.
            """
            nc.vector.memset(vsb[b][:], 1.0)
            for i, sc in enumerate(reversed(range(S // 512))):
                gc = b * S + sc * 512  # global col
                xc = xpool.tile([P, EB, 512], bf16, tag="xc", name="xc")
                if b == 0 and i == 0:
                    # interleave wqk/x per-eb so matmul eb=0 can start
                    # after ~1/8 of the startup bytes; remaining consts
                    # queue behind it off the critical path
                    for eb in range(EB):
                        nc.sync.dma_start(wqk[:, eb, :], wqk_d[:, eb, :])
                        nc.sync.dma_start(xc[:, eb, :],
                                          xT_d[:, eb, gc:gc + 512])
                        if eb == 0:
                            nc.sync.dma_start(bqk[:], bqk_d[:, :])
                    nc.sync.dma_start(wv[:], wv_d[:, :, :])
                    nc.sync.dma_start(bvc[:], bvc_d[:, :])
                    nc.sync.dma_start(tri[:], tri_d[:, :])
                else:
                    nc.sync.dma_start(xc[:], xT_d[:, :, gc:gc + 512])
                for db in range(2):
                    ps = psm.tile([P, 512], f32, tag="mid", name="psqk")
                    for eb in range(EB):
                        nc.tensor.matmul(
                            ps[:],
                            lhsT=wqk[:, eb, db * P:(db + 1) * P],
                            rhs=xc[:, eb, :],
                            start=(eb == 0), stop=(eb == EB - 1),
                        )
                    nc.vector.tensor_scalar_add(
                        qkT[b][:, db, sc * 512:(sc + 1) * 512], ps[:],
                        bqk[:, db:db + 1])
                    yield
                for si in range(4):
                    sb = sc * 4 + si
                    pv_ = psm.tile([P, P], f32, tag="mid", name="psv")
                    for eb in range(EB):
                        nc.tensor.matmul(
                            pv_[:], lhsT=xc[:, eb, si * P:(si + 1) * P],
                            rhs=wv[:, eb, :], start=(eb == 0),
                            stop=(eb == EB - 1))
                    # v bias is NOT added here: attn = (exp@v)/denom + bv
                    # since softmax rows sum to 1 -- bv lands as a cheap
                    # per-partition add after the normalize instead
                    nc.vector.tensor_copy(vsb[b][:, sb, 0, 0:64], pv_[:, 0:64])
                    nc.vector.tensor_copy(vsb[b][:, sb, 1, 0:64], pv_[:, 64:128])
                    yield

        def score_pieces(b, h, expst, order=None):
            """scores^T + exp for one (batch, head), one k-block per piece.

            Default k-block order is high-to-low, matching qkv_pieces'
            suffix-first chunks. Batch 1 uses middle-out ([15..8, 0..7]) so
            its PV q-tiles unlock incrementally as the low k-blocks arrive.
            """
            hs = slice(h * 64, (h + 1) * 64)
            if not expst:
                expst.extend([None] * SBB)
            if order is None:
                order = list(reversed(range(SBB)))
            for kb in order:
                L = S - kb * P
                # 4 bufs: both batches' tiles live concurrently, so
                # batch-1 exps never wait on batch-0's PV to release slots
                et = epool.tile([P, L], bf16, tag=f"e{kb}", name=f"e{kb}",
                                bufs=4)
                off = kb * P
                pos = 0
                while pos < L:  # 1024-wide psum tiles: 1 exp op per tile
                    c = min(1024, L - pos)
                    ps = pbig.tile([P, 1024], f32, tag="big", name="pssc")
                    for c0 in range(0, c, 512):
                        w = min(512, c - c0)
                        nc.tensor.matmul(
                            ps[:, c0:c0 + w],
                            lhsT=qkT[b][hs, 1, off:off + P],
                            rhs=qkT[b][hs, 0, off + pos + c0:off + pos + c0 + w],
                            start=True, stop=True)
                    nc.scalar.activation(
                        et[:, pos:pos + c], ps[:, :c],
                        mybir.ActivationFunctionType.Exp)
                    pos += c
                # zero the invalid (q < k) half of the diagonal block.
                # DVE (not GpSimd): keeps the gpsimd queue empty so the
                # collective triggers fire as soon as their DMAs land.
                nc.vector.tensor_mul(et[:, 0:P], et[:, 0:P], tri[:])
                expst[kb] = et
                yield

        def pv_pieces(b, e0, e1):
            """Flipped PV for batch b: one (q-tile, head) chain per piece.

            out[d, q] = sum_kb vsb[kb]^T @ expst[kb][:, qwin]: N=512 moving
            columns per matmul, stationary operand only 65 columns, so the
            PE stays matmul-bound (no LDWEIGHTS stalls, no HAM cooldown).
            Row 64 accumulates the softmax denominator (ones column of vsb).
            After both heads' chains for a q-tile: reciprocal rows ->
            K=2 broadcast matmul -> two DVE mults write attnT normalized.
            """
            expst = (e0, e1)
            for qt in range(NQT):
                q0 = qt * QT
                pvs = [None, None]
                for h in range(HPC):
                    pp = ppv.tile([65, QT], f32, tag=f"pv{h}",
                                  name=f"pv{h}")
                    nkb = 4 * qt + 4  # k-blocks touching this q-tile
                    for kb in range(nkb):
                        ecol = q0 - kb * P  # expst col of q-tile start
                        poff = max(0, -ecol)
                        w = QT - poff
                        nc.tensor.matmul(
                            pp[:, poff:QT],
                            lhsT=vsb[b][:, kb, h, 0:65],
                            rhs=expst[h][kb][:, ecol + poff:ecol + poff + w],
                            start=(kb == 0), stop=(kb == nkb - 1))
                    # fast-release: one DVE copy frees the PSUM slot so the
                    # next chain never waits on the normalize tail
                    pvs[h] = small.tile([65, QT], bf16, tag=f"pvs{h}",
                                        name=f"pvs{h}", bufs=1)
                    nc.vector.tensor_copy(pvs[h][:], pp[:, :])
                    yield
                # broadcast each raw denominator row across 64 partitions
                # with a K=1 matmul, take the reciprocal on the broadcast
                # (per-lane cost is free-dim-bound, so this costs the same
                # as a single-row reciprocal but needs no extra copy), then
                # normalize into the head's attnT half
                for h in range(HPC):
                    bc = psm.tile([D, QT], f32, tag="mid", name="bc")
                    nc.tensor.matmul(bc[0:D, :],
                                     lhsT=onesc[64:65, 0:D],
                                     rhs=pvs[h][64:65, :],
                                     start=True, stop=True)
                    bcs = small.tile([D, QT], f32, tag="bcs",
                                     name=f"bcs{h}", bufs=1)
                    nc.vector.reciprocal_approx_fast(out=bcs[:], in_=bc[0:D, :])
                    nc.vector.tensor_mul(attnT[b][h][0:D, q0:q0 + QT],
                                         pvs[h][0:D, :], bcs[0:D, :])
                    nc.vector.tensor_scalar_add(
                        attnT[b][h][0:D, q0:q0 + QT],
                        attnT[b][h][0:D, q0:q0 + QT], bvc[0:D, h:h + 1])
                yield

        def interleave(*gens):
            gens = list(gens)
            while gens:
                gens = [g for g in gens if next(g, StopIteration) is not StopIteration]

        def paced(qg, score_gens, pv_gens=(), pv_every=1):
            """Weave one qkv stream with score/pv streams, pacing emission so
            every score k-block is emitted AFTER the qkv chunk that writes the
            qkT columns it reads (Tile only tracks writer->reader deps in
            emission order). qkv chunk g (suffix-first) unlocks score k-blocks
            [12-4g, 15-4g]."""
            rnd = 0
            for g in range(4):
                for _ in range(6):
                    next(qg, None)
                for _ in range(4):
                    for sg in score_gens:
                        next(sg, None)
                    if rnd % pv_every == 0:
                        for pg in pv_gens:
                            next(pg, None)
                    rnd += 1
            interleave(qg, *score_gens, *pv_gens)

        atf = [work.tile([P, EB, 2 * P], bf16, tag="atf",
                         name=f"atf{b}") for b in range(B)]

        def bounce(b):
            """attnT -> a2a_in: chunk j of the bounce gets token blocks
            {j, j+8}; head h's 64 rows land at payload partitions h*64+.
            On the otherwise-empty GpSimd SWDGE queue so the collective
            trigger right behind it fires immediately."""
            for t in range(2):
                for h in range(HPC):
                    nc.gpsimd.dma_start(
                        a2a_in[b].ap().rearrange(
                            "j p (t c) -> p j t c",
                            t=2)[h * D:(h + 1) * D, :, t, :],
                        attnT[b][h][:, t * NCORES * P:(t + 1) * NCORES * P]
                        .rearrange("p (j c) -> p j c", c=P))

        def a2a_batch(b):
            """AllToAll of batch b (512KB per rank)."""
            if no_cc:
                for j in range(NCORES):
                    nc.sync.dma_start(a2a_out[b][j], a2a_in[b][j])
            else:
                nc.gpsimd.collective_compute(
                    "AllToAll", mybir.AluOpType.bypass,
                    replica_groups=[list(range(NCORES))],
                    ins=[a2a_in[b].ap()], outs=[a2a_out[b].ap()])

        def atf_gather(b):
            nc.sync.dma_start(
                atf[b][:, :, :],
                a2a_out[b].ap().rearrange("j p c -> p j c"))

        def oproj_half(b, st):
            """Output projection of token block st*8 + rank of batch b.
            b_o is host-broadcast to all partitions, so the bias rides the
            PSUM->SBUF copyout as a DVE add (no K=1 bias matmuls)."""
            ot = opool.tile([P, E], bf16, tag="o", name="ot")
            po = pbig.tile([P, 1024], f32, tag="big", name="pso")
            for oh in range(2):
                for eb in range(EB):
                    nc.tensor.matmul(
                        po[:, oh * 512:(oh + 1) * 512],
                        lhsT=atf[b][:, eb, st * P:(st + 1) * P],
                        rhs=woh[oh][:, eb, :],
                        start=(eb == 0), stop=(eb == EB - 1))
            nc.vector.tensor_tensor(out=ot[:], in0=po[:], in1=bo[:],
                                    op=mybir.AluOpType.add)
            nc.sync.dma_start(out_d[b * 2 + st], ot[:])

        # ---- pipelined emission (priorities; Tile schedules by readiness) ----
        # Phase-sequential PE stream (in-order engine queues make fine
        # interleaving counterproductive): batch-0 QKV+scores, batch-1
        # QKV+scores (ACT exps trail), then both PV phases back to back --
        # each triggers its half-AllToAlls as attnT halves complete -- and
        # the four output projections last, overlapping the tail collectives.
        e00, e01, e10, e11 = [], [], [], []
        paced(qkv_pieces(0),
              [score_pieces(0, 0, e00), score_pieces(0, 1, e01)])
        nc.sync.dma_start(bo[:, :], bo_d[:, :])
        # batch 1 middle-out: high k-blocks pace with the suffix-first qkv
        # chunks; the low half is emitted only after ALL qkv1 pieces (its
        # matmuls read every qkT column -- emission order must respect
        # writer->reader) and runs forward so pv(1) unlocks incrementally.
        mid_hi = list(reversed(range(8, SBB)))
        mid_lo = list(range(8))
        paced(qkv_pieces(1),
              [score_pieces(1, 0, e10, mid_hi),
               score_pieces(1, 1, e11, mid_hi)])
        # W_o halves land in the two xc slots the moment QKV stops using
        # them (same shape/tag); loaded well before the first oproj
        woh = [xpool.tile([P, EB, 512], bf16, tag="xc", name=f"wo{oh}")
               for oh in range(2)]
        for oh in range(2):
            nc.sync.dma_start(woh[oh][:], wo_d[:, :, oh * 512:(oh + 1) * 512])
        # PV(0) before batch-1's low score blocks: its exps are long done,
        # it's dense PE work, and finishing it here fires A2A(0) ~25us
        # earlier so oproj(0) never waits on it at the tail
        interleave(pv_pieces(0, e00, e01))
        bounce(0)
        a2a_batch(0)            # overlaps scores1lo + batch-1 PV
        interleave(score_pieces(1, 0, e10, mid_lo),
                   score_pieces(1, 1, e11, mid_lo))
        interleave(pv_pieces(1, e10, e11))
        bounce(1)
        a2a_batch(1)            # overlaps oproj of batch 0
        atf_gather(0)
        oproj_half(0, 0)
        oproj_half(0, 1)
        atf_gather(1)
        oproj_half(1, 0)
        oproj_half(1, 1)

    nc.compile()
    return nc


def _in_maps(x, W_qkv, b_qkv, W_o, b_o):
    # [partition, eblock, col] layouts (see dram tensor decls)
    xT = np.ascontiguousarray(
        x.reshape(BS, EB, P).transpose(2, 1, 0)).astype(_bf16)
    wo = np.ascontiguousarray(
        W_o.reshape(EB, P, E).transpose(1, 0, 2)).astype(_bf16)
    bo = np.ascontiguousarray(np.broadcast_to(
        np.asarray(b_o).reshape(1, E), (P, E))).astype(_bf16)
    tri = np.triu(np.ones((P, P), np.float32)).astype(_bf16)
    maps = []
    for c in range(NCORES):
        o = c * HPC * D
        q_sl = slice(o, o + HPC * D)
        k_sl = slice(E + o, E + o + HPC * D)
        v_sl = slice(2 * E + o, 2 * E + o + HPC * D)
        wqk = np.concatenate(
            [W_qkv[:, q_sl] * 0.125, W_qkv[:, k_sl]], axis=1)
        maps.append({
            "xT": xT,
            "wqk": np.ascontiguousarray(
                wqk.reshape(EB, P, 2 * P).transpose(1, 0, 2)).astype(_bf16),
            "wv": np.ascontiguousarray(
                W_qkv[:, v_sl].reshape(EB, P, P).transpose(1, 0, 2)).astype(_bf16),
            "wo": wo,
            "bqk": np.stack([b_qkv[q_sl] * 0.125,
                             b_qkv[k_sl]], axis=1).astype(np.float32),
            "bvc": np.ascontiguousarray(
                b_qkv[v_sl].reshape(HPC, D).T).astype(np.float32),
            "bo": bo,
            "tri": tri,
        })
    return maps


def kernel(x, W_qkv, b_qkv, W_o, b_o, mask):
    from concourse.bass_utils import run_bass_kernel_spmd

    if "nc" not in _cache:
        _cache["nc"] = _build()
    nc = _cache["nc"]
    maps = _in_maps(np.asarray(x, np.float32), np.asarray(W_qkv, np.float32),
                    np.asarray(b_qkv, np.float32), np.asarray(W_o, np.float32),
                    np.asarray(b_o, np.float32))
    res = run_bass_kernel_spmd(nc, maps, list(range(NCORES)))
    # rank r's out[st] is global 128-token block st*8 + r
    full = np.empty((SB, P, E), np.float32)
    for r in range(NCORES):
        full[r::NCORES] = res.results[r]["out"]
    return full.reshape(B, S, E).astype(np.float32)



# revision 22
# speedup vs baseline: 1.0724x; 1.0469x over previous
"""Multi-head causal self-attention (B=2, S=2048, E=1024, H=16) on 8 TRN2 NeuronCores.

Sharding: tensor-parallel over heads (2 heads/core, both batches). Per core:
  - QKV projection for its 2 heads (q^T/k^T transposed layout, v natural;
    the v bias is deferred -- softmax rows sum to 1, so bv is a cheap
    per-partition DVE add after normalization instead of 64 K=1 matmuls)
  - causal flash-style attention, scores computed transposed (k on
    partitions) and exp'd on ACT in 1024-wide PSUM tiles; tri-mask on DVE
  - PV computed FLIPPED (lhsT = [v | ones], rhs = exp(scores^T)): each
    matmul streams 512 q-columns (matmul-bound, no LDWEIGHTS stalls) and
    the result lands pre-transposed [d, q] with the softmax denominator in
    row 64; a K=1 matmul broadcasts the denominator row across 64
    partitions, reciprocal_approx_fast inverts the broadcast, one DVE
    multiply normalizes into per-head attnT halves (all partition-aligned)
  - one 512KB AllToAll per batch, triggered from the otherwise-empty
    GpSimd queue the moment that batch's attnT completes (collectives
    serialize on TOPSP, so fewer, larger A2As beat four half-sized ones),
    then four output projections, each overlapping the next collective.
Emission is phase-sequential (in-order engine queues): qkv0+scores0
(reversed, paced), qkv1+scores1 (middle-out: high k-blocks paced with the
suffix-first chunks, low half forward after qkv1 so pv1 unlocks
incrementally), pv0, pv1, oproj x4. All 16 exp tiles per (batch, head) keep
4 slot buffers so batch-1 exps never wait on batch-0's PV to release them.
Host side only reshapes/slices inputs and concatenates the 8 disjoint row
shards of the output.
"""

import numpy as np
import ml_dtypes

P = 128
B, S, E, H, D = 2, 2048, 1024, 16, 64
NCORES = 8
EB = E // P            # 8 e-blocks
BS = B * S             # 4096 flattened rows
SBB = S // P           # 16 s-blocks per batch
SB = BS // P           # 32 s-blocks global
HPC = H // NCORES      # 2 heads per core
CH = BS // NCORES      # 512 rows owned per core
QT = 512               # q-tile width for the PV phase
NQT = S // QT          # 4 q-tiles per batch

_bf16 = ml_dtypes.bfloat16
_cache = {}


def _build(no_cc=False):
    from contextlib import ExitStack

    import concourse.tile as tile
    from concourse import bacc, mybir

    bf16 = mybir.dt.bfloat16
    f32 = mybir.dt.float32

    nc = bacc.Bacc("TRN2", target_bir_lowering=False, debug=False,
                   num_devices=NCORES)

    # host-side layouts are [partition, eblock, col] so each load is one DMA
    xT_d = nc.dram_tensor("xT", [P, EB, BS], bf16, kind="ExternalInput")
    wqk_d = nc.dram_tensor("wqk", [P, EB, 2 * P], bf16, kind="ExternalInput")
    wv_d = nc.dram_tensor("wv", [P, EB, P], bf16, kind="ExternalInput")
    wo_d = nc.dram_tensor("wo", [P, EB, E], bf16, kind="ExternalInput")
    bqk_d = nc.dram_tensor("bqk", [P, 2], f32, kind="ExternalInput")
    bo_d = nc.dram_tensor("bo", [P, E], bf16, kind="ExternalInput")
    tri_d = nc.dram_tensor("tri", [P, P], bf16, kind="ExternalInput")
    # rank r owns interleaved token blocks {r, r+8, r+16, r+24}: one AllToAll
    # per batch. out row-block st <-> global block st*8 + rank.
    out_d = nc.dram_tensor("out", [4, P, E], bf16, kind="ExternalOutput")
    a2a_in = [nc.dram_tensor(f"a2a_in{b}", [NCORES, P, 2 * P], bf16)
              for b in range(B)]
    a2a_out = [nc.dram_tensor(f"a2a_out{b}", [NCORES, P, 2 * P], bf16)
               for b in range(B)]

    with tile.TileContext(nc) as tc, ExitStack() as ctx:
        consts = ctx.enter_context(tc.tile_pool(name="consts", bufs=1))
        work = ctx.enter_context(tc.tile_pool(name="work", bufs=1))
        xpool = ctx.enter_context(tc.tile_pool(name="xstream", bufs=2))
        epool = ctx.enter_context(tc.tile_pool(name="expst", bufs=2))
        small = ctx.enter_context(tc.tile_pool(name="small", bufs=2))
        opool = ctx.enter_context(tc.tile_pool(name="osb", bufs=1))
        pbig = ctx.enter_context(tc.tile_pool(name="pbig", bufs=2, space="PSUM"))
        ppv = ctx.enter_context(tc.tile_pool(name="ppv", bufs=1, space="PSUM"))
        psm = ctx.enter_context(tc.tile_pool(name="psm", bufs=2, space="PSUM"))

        wqk = consts.tile([P, EB, 2 * P], bf16, tag="wqk")
        wv = consts.tile([P, EB, P], bf16, tag="wv")
        bqk = consts.tile([P, 2], f32, tag="bqk")
        bo = consts.tile([P, E], bf16, tag="bo")
        tri = consts.tile([P, P], bf16, tag="tri")
        ones1 = consts.tile([1, P], bf16, tag="ones1")
        # all-ones column block: row 64 serves as the K=1 stationary
        # operand that broadcasts the denominator row (also at partition 64)
        onesc = consts.tile([P, D], bf16, tag="onesc")

        nc.vector.memset(ones1[:1, :], 1.0)
        nc.vector.memset(onesc[:], 1.0)

        qkT = [work.tile([P, 2, S], bf16, tag=f"qkT{b}", name=f"qkT{b}")
               for b in range(B)]
        vsb = [work.tile([P, SBB, HPC, 66], bf16, tag=f"vsb{b}", name=f"vsb{b}")
               for b in range(B)]
        # per-head attnT halves (both on partitions 0-63): keeps every DVE
        # normalize op partition-aligned; the bounce DMA does the shift of
        # head 1 into partitions 64-127 of the A2A payload
        attnT = [[work.tile([D, S], bf16, tag=f"attnT{h}",
                            name=f"attnT{b}{h}") for h in range(HPC)]
                 for b in range(B)]

        def qkv_pieces(b):
            """QKV projection for batch b, one 512-token chunk per piece.

            Chunks are emitted suffix-first: causal score block kb only needs
            token columns >= kb*128, so late chunks unblock the small k-blocks
            early and ACT (exp) can start before the whole projection is done.
            """
            nc.vector.memset(vsb[b][:], 1.0)
            for i, sc in enumerate(reversed(range(S // 512))):
                gc = b * S + sc * 512  # global col
                xc = xpool.tile([P, EB, 512], bf16, tag="xc", name="xc")
                if b == 0 and i == 0:
                    # interleave wqk/x per-eb so matmul eb=0 can start
                    # after ~1/8 of the startup bytes; remaining consts
                    # queue behind it off the critical path
                    for eb in range(EB):
                        nc.sync.dma_start(wqk[:, eb, :], wqk_d[:, eb, :])
                        nc.sync.dma_start(xc[:, eb, :],
                                          xT_d[:, eb, gc:gc + 512])
                        if eb == 0:
                            nc.sync.dma_start(bqk[:], bqk_d[:, :])
                    nc.sync.dma_start(wv[:], wv_d[:, :, :])
                    nc.sync.dma_start(tri[:], tri_d[:, :])
                else:
                    nc.sync.dma_start(xc[:], xT_d[:, :, gc:gc + 512])
                for db in range(2):
                    ps = psm.tile([P, 512], f32, tag="mid", name="psqk")
                    for eb in range(EB):
                        nc.tensor.matmul(
                            ps[:],
                            lhsT=wqk[:, eb, db * P:(db + 1) * P],
                            rhs=xc[:, eb, :],
                            start=(eb == 0), stop=(eb == EB - 1),
                        )
                    nc.vector.tensor_scalar_add(
                        qkT[b][:, db, sc * 512:(sc + 1) * 512], ps[:],
                        bqk[:, db:db + 1])
                    yield
                for si in range(4):
                    sb = sc * 4 + si
                    pv_ = psm.tile([P, P], f32, tag="mid", name="psv")
                    for eb in range(EB):
                        nc.tensor.matmul(
                            pv_[:], lhsT=xc[:, eb, si * P:(si + 1) * P],
                            rhs=wv[:, eb, :], start=(eb == 0),
                            stop=(eb == EB - 1))
                    # v bias is NOT added here: softmax rows sum to 1, so
                    # bv@W_o folds into b_o host-side (exact); one 3D-AP
                    # copy drops both heads' slices in place
                    nc.vector.tensor_copy(
                        vsb[b][:, sb, :, 0:64],
                        pv_[:].rearrange("p (h d) -> p h d", h=2))
                    yield

        def score_pieces(b, h, expst, order=None):
            """scores^T + exp for one (batch, head), one k-block per piece.

            Default k-block order is high-to-low, matching qkv_pieces'
            suffix-first chunks. Batch 1 uses middle-out ([15..8, 0..7]) so
            its PV q-tiles unlock incrementally as the low k-blocks arrive.
            """
            hs = slice(h * 64, (h + 1) * 64)
            if not expst:
                expst.extend([None] * SBB)
            if order is None:
                order = list(reversed(range(SBB)))
            for kb in order:
                L = S - kb * P
                # 4 bufs: both batches' tiles live concurrently, so
                # batch-1 exps never wait on batch-0's PV to release slots
                et = epool.tile([P, L], bf16, tag=f"e{kb}", name=f"e{kb}",
                                bufs=4)
                off = kb * P
                pos = 0
                while pos < L:  # 1024-wide psum tiles: 1 exp op per tile
                    c = min(1024, L - pos)
                    ps = pbig.tile([P, 1024], f32, tag="big", name="pssc")
                    for c0 in range(0, c, 512):
                        w = min(512, c - c0)
                        nc.tensor.matmul(
                            ps[:, c0:c0 + w],
                            lhsT=qkT[b][hs, 1, off:off + P],
                            rhs=qkT[b][hs, 0, off + pos + c0:off + pos + c0 + w],
                            start=True, stop=True)
                    nc.scalar.activation(
                        et[:, pos:pos + c], ps[:, :c],
                        mybir.ActivationFunctionType.Exp)
                    pos += c
                # zero the invalid (q < k) half of the diagonal block.
                # DVE (not GpSimd): keeps the gpsimd queue empty so the
                # collective triggers fire as soon as their DMAs land.
                nc.vector.tensor_mul(et[:, 0:P], et[:, 0:P], tri[:])
                expst[kb] = et
                yield

        def pv_pieces(b, e0, e1):
            """Flipped PV for batch b: one (q-tile, head) chain per piece.

            out[d, q] = sum_kb vsb[kb]^T @ expst[kb][:, qwin]: N=512 moving
            columns per matmul, stationary operand only 65 columns, so the
            PE stays matmul-bound (no LDWEIGHTS stalls, no HAM cooldown).
            Row 64 accumulates the softmax denominator (ones column of vsb).
            After both heads' chains for a q-tile: reciprocal rows ->
            K=2 broadcast matmul -> two DVE mults write attnT normalized.
            """
            expst = (e0, e1)
            for qt in range(NQT):
                q0 = qt * QT
                pvs = [None, None]
                for h in range(HPC):
                    pp = ppv.tile([65, QT], f32, tag=f"pv{h}",
                                  name=f"pv{h}")
                    nkb = 4 * qt + 4  # k-blocks touching this q-tile
                    for kb in range(nkb):
                        ecol = q0 - kb * P  # expst col of q-tile start
                        poff = max(0, -ecol)
                        w = QT - poff
                        nc.tensor.matmul(
                            pp[:, poff:QT],
                            lhsT=vsb[b][:, kb, h, 0:65],
                            rhs=expst[h][kb][:, ecol + poff:ecol + poff + w],
                            start=(kb == 0), stop=(kb == nkb - 1))
                    # fast-release: one DVE copy frees the PSUM slot so the
                    # next chain never waits on the normalize tail
                    pvs[h] = small.tile([65, QT], bf16, tag=f"pvs{h}",
                                        name=f"pvs{h}", bufs=1)
                    nc.vector.tensor_copy(pvs[h][:], pp[:, :])
                    yield
                # broadcast each raw denominator row across 64 partitions
                # with a K=1 matmul, take the reciprocal on the broadcast
                # (per-lane cost is free-dim-bound, so this costs the same
                # as a single-row reciprocal but needs no extra copy), then
                # normalize into the head's attnT half
                for h in range(HPC):
                    bc = psm.tile([D, QT], f32, tag="mid", name="bc")
                    nc.tensor.matmul(bc[0:D, :],
                                     lhsT=onesc[64:65, 0:D],
                                     rhs=pvs[h][64:65, :],
                                     start=True, stop=True)
                    bcs = small.tile([D, QT], f32, tag="bcs",
                                     name=f"bcs{h}", bufs=1)
                    nc.vector.reciprocal_approx_fast(out=bcs[:], in_=bc[0:D, :])
                    nc.vector.tensor_mul(attnT[b][h][0:D, q0:q0 + QT],
                                         pvs[h][0:D, :], bcs[0:D, :])
                yield

        def interleave(*gens):
            gens = list(gens)
            while gens:
                gens = [g for g in gens if next(g, StopIteration) is not StopIteration]

        def paced(qg, score_gens, pv_gens=(), pv_every=1):
            """Weave one qkv stream with score/pv streams, pacing emission so
            every score k-block is emitted AFTER the qkv chunk that writes the
            qkT columns it reads (Tile only tracks writer->reader deps in
            emission order). qkv chunk g (suffix-first) unlocks score k-blocks
            [12-4g, 15-4g]."""
            rnd = 0
            for g in range(4):
                for _ in range(6):
                    next(qg, None)
                for _ in range(4):
                    for sg in score_gens:
                        next(sg, None)
                    if rnd % pv_every == 0:
                        for pg in pv_gens:
                            next(pg, None)
                    rnd += 1
            interleave(qg, *score_gens, *pv_gens)

        atf = [work.tile([P, EB, 2 * P], bf16, tag="atf",
                         name=f"atf{b}") for b in range(B)]

        def bounce(b):
            """attnT -> a2a_in: chunk j of the bounce gets token blocks
            {j, j+8}; head h's 64 rows land at payload partitions h*64+.
            On the otherwise-empty GpSimd SWDGE queue so the collective
            trigger right behind it fires immediately."""
            for t in range(2):
                for h in range(HPC):
                    nc.gpsimd.dma_start(
                        a2a_in[b].ap().rearrange(
                            "j p (t c) -> p j t c",
                            t=2)[h * D:(h + 1) * D, :, t, :],
                        attnT[b][h][:, t * NCORES * P:(t + 1) * NCORES * P]
                        .rearrange("p (j c) -> p j c", c=P))

        def a2a_batch(b):
            """AllToAll of batch b (512KB per rank)."""
            if no_cc:
                for j in range(NCORES):
                    nc.sync.dma_start(a2a_out[b][j], a2a_in[b][j])
            else:
                nc.gpsimd.collective_compute(
                    "AllToAll", mybir.AluOpType.bypass,
                    replica_groups=[list(range(NCORES))],
                    ins=[a2a_in[b].ap()], outs=[a2a_out[b].ap()])

        def atf_gather(b):
            nc.sync.dma_start(
                atf[b][:, :, :],
                a2a_out[b].ap().rearrange("j p c -> p j c"))

        def oproj_half(b, st):
            """Output projection of token block st*8 + rank of batch b.
            b_o is host-broadcast to all partitions, so the bias rides the
            PSUM->SBUF copyout as a DVE add (no K=1 bias matmuls)."""
            ot = opool.tile([P, E], bf16, tag="o", name="ot")
            po = pbig.tile([P, 1024], f32, tag="big", name="pso")
            for oh in range(2):
                for eb in range(EB):
                    nc.tensor.matmul(
                        po[:, oh * 512:(oh + 1) * 512],
                        lhsT=atf[b][:, eb, st * P:(st + 1) * P],
                        rhs=woh[oh][:, eb, :],
                        start=(eb == 0), stop=(eb == EB - 1))
            nc.vector.tensor_tensor(out=ot[:], in0=po[:], in1=bo[:],
                                    op=mybir.AluOpType.add)
            nc.sync.dma_start(out_d[b * 2 + st], ot[:])

        # ---- pipelined emission (priorities; Tile schedules by readiness) ----
        # Phase-sequential PE stream (in-order engine queues make fine
        # interleaving counterproductive): batch-0 QKV+scores, batch-1
        # QKV+scores (ACT exps trail), then both PV phases back to back --
        # each triggers its half-AllToAlls as attnT halves complete -- and
        # the four output projections last, overlapping the tail collectives.
        e00, e01, e10, e11 = [], [], [], []
        paced(qkv_pieces(0),
              [score_pieces(0, 0, e00), score_pieces(0, 1, e01)])
        nc.sync.dma_start(bo[:, :], bo_d[:, :])
        # batch 1 middle-out: high k-blocks pace with the suffix-first qkv
        # chunks; the low half is emitted only after ALL qkv1 pieces (its
        # matmuls read every qkT column -- emission order must respect
        # writer->reader) and runs forward so pv(1) unlocks incrementally.
        mid_hi = list(reversed(range(8, SBB)))
        mid_lo = list(range(8))
        paced(qkv_pieces(1),
              [score_pieces(1, 0, e10, mid_hi),
               score_pieces(1, 1, e11, mid_hi)])
        # W_o halves land in the two xc slots the moment QKV stops using
        # them (same shape/tag); loaded well before the first oproj
        woh = [xpool.tile([P, EB, 512], bf16, tag="xc", name=f"wo{oh}")
               for oh in range(2)]
        for oh in range(2):
            nc.sync.dma_start(woh[oh][:], wo_d[:, :, oh * 512:(oh + 1) * 512])
        # PV(0) woven WITH batch-1's low score blocks: pv0's exps are long
        # done so its chains fill the PE while scores1lo is ACT-bound, and
        # finishing pv0 here fires A2A(0) ~18us earlier so oproj(0) never
        # waits on it at the tail; scores1lo's exp schedule (and so pv1)
        # is unchanged.
        interleave(score_pieces(1, 0, e10, mid_lo),
                   score_pieces(1, 1, e11, mid_lo),
                   pv_pieces(0, e00, e01))
        bounce(0)
        a2a_batch(0)            # overlaps batch-1 PV
        interleave(pv_pieces(1, e10, e11))
        bounce(1)
        a2a_batch(1)            # overlaps oproj of batch 0
        atf_gather(0)
        oproj_half(0, 0)
        oproj_half(0, 1)
        atf_gather(1)
        oproj_half(1, 0)
        oproj_half(1, 1)

    nc.compile()
    return nc


def _in_maps(x, W_qkv, b_qkv, W_o, b_o):
    # [partition, eblock, col] layouts (see dram tensor decls)
    xT = np.ascontiguousarray(
        x.reshape(BS, EB, P).transpose(2, 1, 0)).astype(_bf16)
    wo = np.ascontiguousarray(
        W_o.reshape(EB, P, E).transpose(1, 0, 2)).astype(_bf16)
    # fold the v bias through the output projection: softmax rows sum to
    # 1, so attn = softmax@v + bv and out = softmax@v@W_o + (bv@W_o + b_o)
    bo2 = np.asarray(b_o, np.float64) + np.asarray(
        b_qkv[2 * E:], np.float64) @ np.asarray(W_o, np.float64)
    bo = np.ascontiguousarray(np.broadcast_to(
        bo2.reshape(1, E), (P, E))).astype(_bf16)
    tri = np.triu(np.ones((P, P), np.float32)).astype(_bf16)
    maps = []
    for c in range(NCORES):
        o = c * HPC * D
        q_sl = slice(o, o + HPC * D)
        k_sl = slice(E + o, E + o + HPC * D)
        v_sl = slice(2 * E + o, 2 * E + o + HPC * D)
        wqk = np.concatenate(
            [W_qkv[:, q_sl] * 0.125, W_qkv[:, k_sl]], axis=1)
        maps.append({
            "xT": xT,
            "wqk": np.ascontiguousarray(
                wqk.reshape(EB, P, 2 * P).transpose(1, 0, 2)).astype(_bf16),
            "wv": np.ascontiguousarray(
                W_qkv[:, v_sl].reshape(EB, P, P).transpose(1, 0, 2)).astype(_bf16),
            "wo": wo,
            "bqk": np.stack([b_qkv[q_sl] * 0.125,
                             b_qkv[k_sl]], axis=1).astype(np.float32),
            "bo": bo,
            "tri": tri,
        })
    return maps


def kernel(x, W_qkv, b_qkv, W_o, b_o, mask):
    from concourse.bass_utils import run_bass_kernel_spmd

    if "nc" not in _cache:
        _cache["nc"] = _build()
    nc = _cache["nc"]
    maps = _in_maps(np.asarray(x, np.float32), np.asarray(W_qkv, np.float32),
                    np.asarray(b_qkv, np.float32), np.asarray(W_o, np.float32),
                    np.asarray(b_o, np.float32))
    res = run_bass_kernel_spmd(nc, maps, list(range(NCORES)))
    # rank r's out[st] is global 128-token block st*8 + r
    full = np.empty((SB, P, E), np.float32)
    for r in range(NCORES):
        full[r::NCORES] = res.results[r]["out"]
    return full.reshape(B, S, E).astype(np.float32)



# revision 25
# speedup vs baseline: 1.1466x; 1.0693x over previous
"""Multi-head causal self-attention (B=2, S=2048, E=1024, H=16) on 8 TRN2 NeuronCores.

Sharding: tensor-parallel over heads (2 heads/core, both batches). Per core:
  - QKV projection for its 2 heads (q^T/k^T transposed layout, v natural;
    both biases are folded away: softmax rows sum to 1, so bv rides
    through the output projection as bo' = bo + bv@W_o, computed host-side)
  - causal flash-style attention, scores computed transposed (k on
    partitions) and exp'd on ACT in 1024-wide PSUM tiles; tri-mask on DVE
  - PV computed FLIPPED (lhsT = [v | ones], rhs = exp(scores^T)): each
    matmul streams 512 q-columns (matmul-bound, no LDWEIGHTS stalls) and
    the result lands pre-transposed [d, q] with the softmax denominator in
    row 64; a K=1 matmul broadcasts the denominator row across 64
    partitions, reciprocal_approx_fast inverts the broadcast, one DVE
    multiply normalizes into per-head attnT halves (all partition-aligned)
  - one 512KB AllToAll per batch, triggered from the otherwise-empty
    GpSimd queue the moment that batch's attnT completes (collectives
    have a ~13us fixed cost, so fewer, larger A2As beat four half-sized
    ones), then four output projections (bias added by the DVE copyout
    against a host-broadcast bo'), each overlapping the next collective.
Emission is phase-sequential (matching HAM clock-gate behavior: dense PE
phases stay at 8/8): qkv0+scores0 (reversed, paced), qkv1+scores1
(middle-out: high k-blocks paced with the suffix-first chunks), then the
batch-1 low score blocks INTERLEAVED with pv0 (pv0's exps are long done,
so its chains fill the PE while scores1lo is ACT-bound, and A2A(0) fires
~18us earlier -- early enough that oproj(0) never waits on it), pv1,
A2A(1), oproj x4 (bf16 out DMAs). Startup interleaves wqk/x chunk-0 loads
per e-block so the first matmul starts after ~1/8 of the startup bytes.
All 16 exp tiles per (batch, head) keep 4 slot buffers so batch-1 exps
never wait on batch-0's PV to release them. Host side only reshapes/
slices inputs and concatenates the 8 disjoint row shards of the output.
"""

import numpy as np
import ml_dtypes

P = 128
B, S, E, H, D = 2, 2048, 1024, 16, 64
NCORES = 8
EB = E // P            # 8 e-blocks
BS = B * S             # 4096 flattened rows
SBB = S // P           # 16 s-blocks per batch
SB = BS // P           # 32 s-blocks global
HPC = H // NCORES      # 2 heads per core
CH = BS // NCORES      # 512 rows owned per core
QT = 512               # q-tile width for the PV phase
NQT = S // QT          # 4 q-tiles per batch

_bf16 = ml_dtypes.bfloat16
_cache = {}


def _build(no_cc=False):
    from contextlib import ExitStack

    import concourse.tile as tile
    from concourse import bacc, mybir

    bf16 = mybir.dt.bfloat16
    f32 = mybir.dt.float32

    nc = bacc.Bacc("TRN2", target_bir_lowering=False, debug=False,
                   num_devices=NCORES)

    # host-side layouts are [partition, eblock, col] so each load is one DMA
    xT_d = nc.dram_tensor("xT", [P, EB, BS], bf16, kind="ExternalInput")
    wqk_d = nc.dram_tensor("wqk", [P, EB, 2 * P], bf16, kind="ExternalInput")
    wv_d = nc.dram_tensor("wv", [P, EB, P], bf16, kind="ExternalInput")
    wo_d = nc.dram_tensor("wo", [P, EB, E], bf16, kind="ExternalInput")
    bqk_d = nc.dram_tensor("bqk", [P, 2], f32, kind="ExternalInput")
    bo_d = nc.dram_tensor("bo", [P, E], bf16, kind="ExternalInput")
    tri_d = nc.dram_tensor("tri", [P, P], bf16, kind="ExternalInput")
    # rank r owns interleaved token blocks {r, r+8, r+16, r+24}: one AllToAll
    # per batch. out row-block st <-> global block st*8 + rank.
    out_d = nc.dram_tensor("out", [4, P, E], bf16, kind="ExternalOutput")
    a2a_in = [nc.dram_tensor(f"a2a_in{b}", [NCORES, P, 2 * P], bf16)
              for b in range(B)]
    a2a_out = [nc.dram_tensor(f"a2a_out{b}", [NCORES, P, 2 * P], bf16)
               for b in range(B)]

    with tile.TileContext(nc) as tc, ExitStack() as ctx:
        consts = ctx.enter_context(tc.tile_pool(name="consts", bufs=1))
        work = ctx.enter_context(tc.tile_pool(name="work", bufs=1))
        xpool = ctx.enter_context(tc.tile_pool(name="xstream", bufs=2))
        epool = ctx.enter_context(tc.tile_pool(name="expst", bufs=2))
        small = ctx.enter_context(tc.tile_pool(name="small", bufs=2))
        opool = ctx.enter_context(tc.tile_pool(name="osb", bufs=1))
        pbig = ctx.enter_context(tc.tile_pool(name="pbig", bufs=2, space="PSUM"))
        ppv = ctx.enter_context(tc.tile_pool(name="ppv", bufs=1, space="PSUM"))
        psm = ctx.enter_context(tc.tile_pool(name="psm", bufs=2, space="PSUM"))

        wqk = consts.tile([P, EB, 2 * P], bf16, tag="wqk")
        wv = consts.tile([P, EB, P], bf16, tag="wv")
        bqk = consts.tile([P, 2], f32, tag="bqk")
        bo = consts.tile([P, E], bf16, tag="bo")
        tri = consts.tile([P, P], bf16, tag="tri")
        ones1 = consts.tile([1, P], bf16, tag="ones1")
        # all-ones column block: row 64 serves as the K=1 stationary
        # operand that broadcasts the denominator row (also at partition 64)
        onesc = consts.tile([P, D], bf16, tag="onesc")

        nc.vector.memset(ones1[:1, :], 1.0)
        nc.vector.memset(onesc[:], 1.0)

        qkT = [work.tile([P, 2, S], bf16, tag=f"qkT{b}", name=f"qkT{b}")
               for b in range(B)]
        vsb = [work.tile([P, SBB, HPC, 66], bf16, tag=f"vsb{b}", name=f"vsb{b}")
               for b in range(B)]
        # per-head attnT halves (both on partitions 0-63): keeps every DVE
        # normalize op partition-aligned; the bounce DMA does the shift of
        # head 1 into partitions 64-127 of the A2A payload
        attnT = [[work.tile([D, S], bf16, tag=f"attnT{h}",
                            name=f"attnT{b}{h}") for h in range(HPC)]
                 for b in range(B)]

        def qkv_pieces(b):
            """QKV projection for batch b, one 512-token chunk per piece.

            Chunks are emitted suffix-first: causal score block kb only needs
            token columns >= kb*128, so late chunks unblock the small k-blocks
            early and ACT (exp) can start before the whole projection is done.
            """
            nc.vector.memset(vsb[b][:], 1.0)
            for i, sc in enumerate(reversed(range(S // 512))):
                gc = b * S + sc * 512  # global col
                xc = xpool.tile([P, EB, 512], bf16, tag="xc", name="xc")
                if b == 0 and i == 0:
                    # interleave wqk/x per-eb so matmul eb=0 can start
                    # after ~1/8 of the startup bytes; remaining consts
                    # queue behind it off the critical path
                    for eb in range(EB):
                        nc.sync.dma_start(wqk[:, eb, :], wqk_d[:, eb, :])
                        nc.sync.dma_start(xc[:, eb, :],
                                          xT_d[:, eb, gc:gc + 512])
                        if eb == 0:
                            nc.sync.dma_start(bqk[:], bqk_d[:, :])
                    nc.sync.dma_start(wv[:], wv_d[:, :, :])
                    nc.sync.dma_start(tri[:], tri_d[:, :])
                else:
                    nc.sync.dma_start(xc[:], xT_d[:, :, gc:gc + 512])
                for db in range(2):
                    ps = psm.tile([P, 512], f32, tag="mid", name="psqk")
                    for eb in range(EB):
                        nc.tensor.matmul(
                            ps[:],
                            lhsT=wqk[:, eb, db * P:(db + 1) * P],
                            rhs=xc[:, eb, :],
                            start=(eb == 0), stop=(eb == EB - 1),
                        )
                    nc.vector.tensor_scalar_add(
                        qkT[b][:, db, sc * 512:(sc + 1) * 512], ps[:],
                        bqk[:, db:db + 1])
                    yield
                for si in range(4):
                    sb = sc * 4 + si
                    pv_ = psm.tile([P, P], f32, tag="mid", name="psv")
                    for eb in range(EB):
                        nc.tensor.matmul(
                            pv_[:], lhsT=xc[:, eb, si * P:(si + 1) * P],
                            rhs=wv[:, eb, :], start=(eb == 0),
                            stop=(eb == EB - 1))
                    # v bias is NOT added here: softmax rows sum to 1, so
                    # bv@W_o folds into b_o host-side (exact); one 3D-AP
                    # copy drops both heads' slices in place
                    nc.vector.tensor_copy(
                        vsb[b][:, sb, :, 0:64],
                        pv_[:].rearrange("p (h d) -> p h d", h=2))
                    yield

        def score_pieces(b, h, expst, order=None):
            """scores^T + exp for one (batch, head), one k-block per piece.

            Default k-block order is high-to-low, matching qkv_pieces'
            suffix-first chunks. Batch 1 uses middle-out ([15..8, 0..7]) so
            its PV q-tiles unlock incrementally as the low k-blocks arrive.
            """
            hs = slice(h * 64, (h + 1) * 64)
            if not expst:
                expst.extend([None] * SBB)
            if order is None:
                order = list(reversed(range(SBB)))
            for kb in order:
                L = S - kb * P
                # 4 bufs: both batches' tiles live concurrently, so
                # batch-1 exps never wait on batch-0's PV to release slots
                et = epool.tile([P, L], bf16, tag=f"e{kb}", name=f"e{kb}",
                                bufs=4)
                off = kb * P
                pos = 0
                while pos < L:  # 1024-wide psum tiles: 1 exp op per tile
                    c = min(1024, L - pos)
                    ps = pbig.tile([P, 1024], f32, tag="big", name="pssc")
                    for c0 in range(0, c, 512):
                        w = min(512, c - c0)
                        nc.tensor.matmul(
                            ps[:, c0:c0 + w],
                            lhsT=qkT[b][hs, 1, off:off + P],
                            rhs=qkT[b][hs, 0, off + pos + c0:off + pos + c0 + w],
                            start=True, stop=True)
                    nc.scalar.activation(
                        et[:, pos:pos + c], ps[:, :c],
                        mybir.ActivationFunctionType.Exp)
                    pos += c
                # zero the invalid (q < k) half of the diagonal block.
                # DVE (not GpSimd): keeps the gpsimd queue empty so the
                # collective triggers fire as soon as their DMAs land.
                nc.vector.tensor_mul(et[:, 0:P], et[:, 0:P], tri[:])
                expst[kb] = et
                yield

        def pv_pieces(b, e0, e1):
            """Flipped PV for batch b: one (q-tile, head) chain per piece.

            out[d, q] = sum_kb vsb[kb]^T @ expst[kb][:, qwin]: N=512 moving
            columns per matmul, stationary operand only 65 columns, so the
            PE stays matmul-bound (no LDWEIGHTS stalls, no HAM cooldown).
            Row 64 accumulates the softmax denominator (ones column of vsb).
            After both heads' chains for a q-tile: reciprocal rows ->
            K=2 broadcast matmul -> two DVE mults write attnT normalized.
            """
            expst = (e0, e1)
            for qt in range(NQT):
                q0 = qt * QT
                pvs = [None, None]
                for h in range(HPC):
                    pp = ppv.tile([65, QT], f32, tag=f"pv{h}",
                                  name=f"pv{h}")
                    nkb = 4 * qt + 4  # k-blocks touching this q-tile
                    for kb in range(nkb):
                        ecol = q0 - kb * P  # expst col of q-tile start
                        poff = max(0, -ecol)
                        w = QT - poff
                        nc.tensor.matmul(
                            pp[:, poff:QT],
                            lhsT=vsb[b][:, kb, h, 0:65],
                            rhs=expst[h][kb][:, ecol + poff:ecol + poff + w],
                            start=(kb == 0), stop=(kb == nkb - 1))
                    # fast-release: one DVE copy frees the PSUM slot so the
                    # next chain never waits on the normalize tail
                    pvs[h] = small.tile([65, QT], bf16, tag=f"pvs{h}",
                                        name=f"pvs{h}", bufs=1)
                    nc.vector.tensor_copy(pvs[h][:], pp[:, :])
                    yield
                # broadcast each raw denominator row across 64 partitions
                # with a K=1 matmul, take the reciprocal on the broadcast
                # (per-lane cost is free-dim-bound, so this costs the same
                # as a single-row reciprocal but needs no extra copy), then
                # normalize into the head's attnT half
                for h in range(HPC):
                    bc = psm.tile([D, QT], f32, tag="mid", name="bc")
                    nc.tensor.matmul(bc[0:D, :],
                                     lhsT=onesc[64:65, 0:D],
                                     rhs=pvs[h][64:65, :],
                                     start=True, stop=True)
                    bcs = small.tile([D, QT], f32, tag="bcs",
                                     name=f"bcs{h}", bufs=1)
                    nc.vector.reciprocal_approx_fast(out=bcs[:],
                                                     in_=bc[0:D, :])
                    nc.vector.tensor_mul(attnT[b][h][0:D, q0:q0 + QT],
                                         pvs[h][0:D, :], bcs[0:D, :])
                yield

        def interleave(*gens):
            gens = list(gens)
            while gens:
                gens = [g for g in gens if next(g, StopIteration) is not StopIteration]

        def paced(qg, score_gens, pv_gens=(), pv_every=1):
            """Weave one qkv stream with score/pv streams, pacing emission so
            every score k-block is emitted AFTER the qkv chunk that writes the
            qkT columns it reads (Tile only tracks writer->reader deps in
            emission order). qkv chunk g (suffix-first) unlocks score k-blocks
            [12-4g, 15-4g]."""
            rnd = 0
            for g in range(4):
                for _ in range(6):
                    next(qg, None)
                for _ in range(4):
                    for sg in score_gens:
                        next(sg, None)
                    if rnd % pv_every == 0:
                        for pg in pv_gens:
                            next(pg, None)
                    rnd += 1
            interleave(qg, *score_gens, *pv_gens)

        atf = [work.tile([P, EB, 2 * P], bf16, tag="atf",
                         name=f"atf{b}") for b in range(B)]

        def bounce(b):
            """attnT -> a2a_in: chunk j of the bounce gets token blocks
            {j, j+8}; head h's 64 rows land at payload partitions h*64+.
            On the otherwise-empty GpSimd SWDGE queue so the collective
            trigger right behind it fires immediately."""
            for t in range(2):
                for h in range(HPC):
                    nc.gpsimd.dma_start(
                        a2a_in[b].ap().rearrange(
                            "j p (t c) -> p j t c",
                            t=2)[h * D:(h + 1) * D, :, t, :],
                        attnT[b][h][:, t * NCORES * P:(t + 1) * NCORES * P]
                        .rearrange("p (j c) -> p j c", c=P))

        def a2a_batch(b):
            """AllToAll of batch b (512KB per rank)."""
            if no_cc:
                for j in range(NCORES):
                    nc.sync.dma_start(a2a_out[b][j], a2a_in[b][j])
            else:
                nc.gpsimd.collective_compute(
                    "AllToAll", mybir.AluOpType.bypass,
                    replica_groups=[list(range(NCORES))],
                    ins=[a2a_in[b].ap()], outs=[a2a_out[b].ap()])

        def atf_gather(b):
            nc.sync.dma_start(
                atf[b][:, :, :],
                a2a_out[b].ap().rearrange("j p c -> p j c"))

        def oproj_half(b, st):
            """Output projection of token block st*8 + rank of batch b.
            b_o is host-broadcast to all partitions, so the bias rides the
            PSUM->SBUF copyout as a DVE add (no K=1 bias matmuls)."""
            ot = opool.tile([P, E], bf16, tag="o", name="ot")
            po = pbig.tile([P, 1024], f32, tag="big", name="pso")
            for oh in range(2):
                for eb in range(EB):
                    nc.tensor.matmul(
                        po[:, oh * 512:(oh + 1) * 512],
                        lhsT=atf[b][:, eb, st * P:(st + 1) * P],
                        rhs=woh[oh][:, eb, :],
                        start=(eb == 0), stop=(eb == EB - 1))
            nc.vector.tensor_tensor(out=ot[:], in0=po[:], in1=bo[:],
                                    op=mybir.AluOpType.add)
            nc.sync.dma_start(out_d[b * 2 + st], ot[:])

        # ---- pipelined emission (priorities; Tile schedules by readiness) ----
        # Phase-sequential PE stream (in-order engine queues make fine
        # interleaving counterproductive): batch-0 QKV+scores, batch-1
        # QKV+scores (ACT exps trail), then both PV phases back to back --
        # each triggers its half-AllToAlls as attnT halves complete -- and
        # the four output projections last, overlapping the tail collectives.
        e00, e01, e10, e11 = [], [], [], []
        paced(qkv_pieces(0),
              [score_pieces(0, 0, e00), score_pieces(0, 1, e01)])
        nc.sync.dma_start(bo[:, :], bo_d[:, :])
        # batch 1 middle-out: high k-blocks pace with the suffix-first qkv
        # chunks; the low half is emitted only after ALL qkv1 pieces (its
        # matmuls read every qkT column -- emission order must respect
        # writer->reader) and runs forward so pv(1) unlocks incrementally.
        mid_hi = list(reversed(range(8, SBB)))
        mid_lo = list(range(8))
        paced(qkv_pieces(1),
              [score_pieces(1, 0, e10, mid_hi),
               score_pieces(1, 1, e11, mid_hi)])
        # W_o halves land in the two xc slots the moment QKV stops using
        # them (same shape/tag); loaded well before the first oproj
        woh = [xpool.tile([P, EB, 512], bf16, tag="xc", name=f"wo{oh}")
               for oh in range(2)]
        for oh in range(2):
            nc.sync.dma_start(woh[oh][:], wo_d[:, :, oh * 512:(oh + 1) * 512])
        # PV(0) woven WITH batch-1's low score blocks: pv0's exps are long
        # done so its chains fill the PE while scores1lo is ACT-bound, and
        # finishing pv0 here fires A2A(0) ~18us earlier so oproj(0) never
        # waits on it at the tail; scores1lo's exp schedule (and so pv1)
        # is unchanged.
        s1lo = [score_pieces(1, 0, e10, mid_lo),
                score_pieces(1, 1, e11, mid_lo)]
        p0 = pv_pieces(0, e00, e01)
        for _ in range(8):
            for sg in s1lo:
                next(sg, None)
            next(p0, None)
            next(p0, None)
        interleave(p0, *s1lo)
        bounce(0)
        a2a_batch(0)            # overlaps batch-1 PV
        interleave(pv_pieces(1, e10, e11))
        bounce(1)
        a2a_batch(1)            # overlaps oproj of batch 0
        atf_gather(0)
        atf_gather(1)
        oproj_half(0, 0)
        oproj_half(0, 1)
        oproj_half(1, 0)
        oproj_half(1, 1)

    nc.compile()
    return nc


def _in_maps(x, W_qkv, b_qkv, W_o, b_o):
    # [partition, eblock, col] layouts (see dram tensor decls)
    xT = np.ascontiguousarray(
        x.reshape(BS, EB, P).transpose(2, 1, 0)).astype(_bf16)
    wo = np.ascontiguousarray(
        W_o.reshape(EB, P, E).transpose(1, 0, 2)).astype(_bf16)
    # fold the v bias through the output projection: softmax rows sum to
    # 1, so attn = softmax@v + bv and out = softmax@v@W_o + (bv@W_o + b_o)
    bo2 = np.asarray(b_o, np.float64) + np.asarray(
        b_qkv[2 * E:], np.float64) @ np.asarray(W_o, np.float64)
    bo = np.ascontiguousarray(np.broadcast_to(
        bo2.reshape(1, E), (P, E))).astype(_bf16)
    tri = np.triu(np.ones((P, P), np.float32)).astype(_bf16)
    maps = []
    for c in range(NCORES):
        o = c * HPC * D
        q_sl = slice(o, o + HPC * D)
        k_sl = slice(E + o, E + o + HPC * D)
        v_sl = slice(2 * E + o, 2 * E + o + HPC * D)
        wqk = np.concatenate(
            [W_qkv[:, q_sl] * 0.125, W_qkv[:, k_sl]], axis=1)
        maps.append({
            "xT": xT,
            "wqk": np.ascontiguousarray(
                wqk.reshape(EB, P, 2 * P).transpose(1, 0, 2)).astype(_bf16),
            "wv": np.ascontiguousarray(
                W_qkv[:, v_sl].reshape(EB, P, P).transpose(1, 0, 2)).astype(_bf16),
            "wo": wo,
            "bqk": np.stack([b_qkv[q_sl] * 0.125,
                             b_qkv[k_sl]], axis=1).astype(np.float32),
            "bo": bo,
            "tri": tri,
        })
    return maps


def kernel(x, W_qkv, b_qkv, W_o, b_o, mask):
    from concourse.bass_utils import run_bass_kernel_spmd

    if "nc" not in _cache:
        _cache["nc"] = _build()
    nc = _cache["nc"]
    maps = _in_maps(np.asarray(x, np.float32), np.asarray(W_qkv, np.float32),
                    np.asarray(b_qkv, np.float32), np.asarray(W_o, np.float32),
                    np.asarray(b_o, np.float32))
    res = run_bass_kernel_spmd(nc, maps, list(range(NCORES)))
    # rank r's out[st] is global 128-token block st*8 + r
    full = np.empty((SB, P, E), np.float32)
    for r in range(NCORES):
        full[r::NCORES] = res.results[r]["out"]
    return full.reshape(B, S, E).astype(np.float32)



# revision 26
# speedup vs baseline: 1.1532x; 1.0057x over previous
"""Multi-head causal self-attention (B=2, S=2048, E=1024, H=16) on 8 TRN2 NeuronCores.

Sharding: tensor-parallel over heads (2 heads/core, both batches). Per core:
  - QKV projection for its 2 heads (q^T/k^T transposed layout, v natural;
    both biases are folded away: softmax rows sum to 1, so bv rides
    through the output projection as bo' = bo + bv@W_o, computed host-side)
  - causal flash-style attention, scores computed transposed (k on
    partitions) and exp'd on ACT in 1024-wide PSUM tiles; tri-mask on DVE
  - PV computed FLIPPED (lhsT = [v | ones], rhs = exp(scores^T)): each
    matmul streams 512 q-columns (matmul-bound, no LDWEIGHTS stalls) and
    the result lands pre-transposed [d, q] with the softmax denominator in
    row 64; a K=1 matmul broadcasts the denominator row across 64
    partitions, reciprocal_approx_fast inverts the broadcast, one DVE
    multiply normalizes into per-head attnT halves (all partition-aligned)
  - one 512KB AllToAll per batch, triggered from the otherwise-empty
    GpSimd queue the moment that batch's attnT completes (collectives
    have a ~13us fixed cost, so fewer, larger A2As beat four half-sized
    ones), then four output projections (bias added by the DVE copyout
    against a host-broadcast bo'), each overlapping the next collective.
Emission is phase-sequential (matching HAM clock-gate behavior: dense PE
phases stay at 8/8): qkv0+scores0 (reversed, paced), qkv1+scores1
(middle-out: high k-blocks paced with the suffix-first chunks), then the
batch-1 low score blocks INTERLEAVED with pv0 (pv0's exps are long done,
so its chains fill the PE while scores1lo is ACT-bound, and A2A(0) fires
~18us earlier -- early enough that oproj(0) never waits on it), pv1,
A2A(1), oproj x4 (bf16 out DMAs). Startup interleaves wqk/x chunk-0 loads
per e-block so the first matmul starts after ~1/8 of the startup bytes.
All 16 exp tiles per (batch, head) keep 4 slot buffers so batch-1 exps
never wait on batch-0's PV to release them. Host side only reshapes/
slices inputs and concatenates the 8 disjoint row shards of the output.
"""

import numpy as np
import ml_dtypes

P = 128
B, S, E, H, D = 2, 2048, 1024, 16, 64
NCORES = 8
EB = E // P            # 8 e-blocks
BS = B * S             # 4096 flattened rows
SBB = S // P           # 16 s-blocks per batch
SB = BS // P           # 32 s-blocks global
HPC = H // NCORES      # 2 heads per core
CH = BS // NCORES      # 512 rows owned per core
QT = 512               # q-tile width for the PV phase
NQT = S // QT          # 4 q-tiles per batch

_bf16 = ml_dtypes.bfloat16
_cache = {}


def _build(no_cc=False):
    from contextlib import ExitStack

    import concourse.tile as tile
    from concourse import bacc, mybir

    bf16 = mybir.dt.bfloat16
    f32 = mybir.dt.float32

    nc = bacc.Bacc("TRN2", target_bir_lowering=False, debug=False,
                   num_devices=NCORES)

    # host-side layouts are [partition, eblock, col] so each load is one DMA
    xT_d = nc.dram_tensor("xT", [P, EB, BS], bf16, kind="ExternalInput")
    wqk_d = nc.dram_tensor("wqk", [P, EB, 2 * P], bf16, kind="ExternalInput")
    wv_d = nc.dram_tensor("wv", [P, EB, P], bf16, kind="ExternalInput")
    wo_d = nc.dram_tensor("wo", [P, EB, E], bf16, kind="ExternalInput")
    bqk_d = nc.dram_tensor("bqk", [P, 2], f32, kind="ExternalInput")
    bo_d = nc.dram_tensor("bo", [P, E], bf16, kind="ExternalInput")
    tri_d = nc.dram_tensor("tri", [P, P], bf16, kind="ExternalInput")
    # rank r owns interleaved token blocks {r, r+8, r+16, r+24}: one AllToAll
    # per batch. out row-block st <-> global block st*8 + rank.
    out_d = nc.dram_tensor("out", [4, P, E], bf16, kind="ExternalOutput")
    a2a_in = [nc.dram_tensor(f"a2a_in{b}", [NCORES, P, 2 * P], bf16)
              for b in range(B)]
    a2a_out = [nc.dram_tensor(f"a2a_out{b}", [NCORES, P, 2 * P], bf16)
               for b in range(B)]

    with tile.TileContext(nc) as tc, ExitStack() as ctx:
        consts = ctx.enter_context(tc.tile_pool(name="consts", bufs=1))
        work = ctx.enter_context(tc.tile_pool(name="work", bufs=1))
        xpool = ctx.enter_context(tc.tile_pool(name="xstream", bufs=2))
        epool = ctx.enter_context(tc.tile_pool(name="expst", bufs=2))
        small = ctx.enter_context(tc.tile_pool(name="small", bufs=2))
        opool = ctx.enter_context(tc.tile_pool(name="osb", bufs=1))
        pbig = ctx.enter_context(tc.tile_pool(name="pbig", bufs=2, space="PSUM"))
        ppv = ctx.enter_context(tc.tile_pool(name="ppv", bufs=1, space="PSUM"))
        psm = ctx.enter_context(tc.tile_pool(name="psm", bufs=2, space="PSUM"))

        wqk = consts.tile([P, EB, 2 * P], bf16, tag="wqk")
        wv = consts.tile([P, EB, P], bf16, tag="wv")
        bqk = consts.tile([P, 2], f32, tag="bqk")
        bo = consts.tile([P, E], bf16, tag="bo")
        tri = consts.tile([P, P], bf16, tag="tri")
        ones1 = consts.tile([1, P], bf16, tag="ones1")
        # all-ones column block: row 64 serves as the K=1 stationary
        # operand that broadcasts the denominator row (also at partition 64)
        onesc = consts.tile([P, D], bf16, tag="onesc")

        nc.vector.memset(ones1[:1, :], 1.0)
        nc.vector.memset(onesc[:], 1.0)

        qkT = [work.tile([P, 2, S], bf16, tag=f"qkT{b}", name=f"qkT{b}")
               for b in range(B)]
        vsb = [work.tile([P, SBB, HPC, 66], bf16, tag=f"vsb{b}", name=f"vsb{b}")
               for b in range(B)]
        # per-head attnT halves (both on partitions 0-63): keeps every DVE
        # normalize op partition-aligned; the bounce DMA does the shift of
        # head 1 into partitions 64-127 of the A2A payload
        attnT = [[work.tile([D, S], bf16, tag=f"attnT{h}",
                            name=f"attnT{b}{h}") for h in range(HPC)]
                 for b in range(B)]

        def qkv_pieces(b):
            """QKV projection for batch b, one 512-token chunk per piece.

            Chunks are emitted suffix-first: causal score block kb only needs
            token columns >= kb*128, so late chunks unblock the small k-blocks
            early and ACT (exp) can start before the whole projection is done.
            """
            nc.vector.memset(vsb[b][:], 1.0)
            for i, sc in enumerate(reversed(range(S // 512))):
                gc = b * S + sc * 512  # global col
                xc = xpool.tile([P, EB, 512], bf16, tag="xc", name="xc")
                if b == 0 and i == 0:
                    # interleave wqk/x per-eb so matmul eb=0 can start
                    # after ~1/8 of the startup bytes; remaining consts
                    # queue behind it off the critical path
                    for eb in range(EB):
                        nc.sync.dma_start(wqk[:, eb, :], wqk_d[:, eb, :])
                        nc.sync.dma_start(xc[:, eb, :],
                                          xT_d[:, eb, gc:gc + 512])
                        if eb == 0:
                            nc.sync.dma_start(bqk[:], bqk_d[:, :])
                    nc.sync.dma_start(wv[:], wv_d[:, :, :])
                    nc.sync.dma_start(tri[:], tri_d[:, :])
                else:
                    nc.sync.dma_start(xc[:], xT_d[:, :, gc:gc + 512])
                for db in range(2):
                    ps = psm.tile([P, 512], f32, tag="mid", name="psqk")
                    for eb in range(EB):
                        nc.tensor.matmul(
                            ps[:],
                            lhsT=wqk[:, eb, db * P:(db + 1) * P],
                            rhs=xc[:, eb, :],
                            start=(eb == 0), stop=(eb == EB - 1),
                        )
                    nc.vector.tensor_scalar_add(
                        qkT[b][:, db, sc * 512:(sc + 1) * 512], ps[:],
                        bqk[:, db:db + 1])
                    yield
                for si in range(4):
                    sb = sc * 4 + si
                    pv_ = psm.tile([P, P], f32, tag="mid", name="psv")
                    for eb in range(EB):
                        nc.tensor.matmul(
                            pv_[:], lhsT=xc[:, eb, si * P:(si + 1) * P],
                            rhs=wv[:, eb, :], start=(eb == 0),
                            stop=(eb == EB - 1))
                    # v bias is NOT added here: softmax rows sum to 1, so
                    # bv@W_o folds into b_o host-side (exact); one 3D-AP
                    # copy drops both heads' slices in place
                    nc.vector.tensor_copy(
                        vsb[b][:, sb, :, 0:64],
                        pv_[:].rearrange("p (h d) -> p h d", h=2))
                    yield

        def score_pieces(b, h, expst, order=None):
            """scores^T + exp for one (batch, head), one k-block per piece.

            Default k-block order is high-to-low, matching qkv_pieces'
            suffix-first chunks. Batch 1 uses middle-out ([15..8, 0..7]) so
            its PV q-tiles unlock incrementally as the low k-blocks arrive.
            """
            hs = slice(h * 64, (h + 1) * 64)
            if not expst:
                expst.extend([None] * SBB)
            if order is None:
                order = list(reversed(range(SBB)))
            for kb in order:
                L = S - kb * P
                # 4 bufs: both batches' tiles live concurrently, so
                # batch-1 exps never wait on batch-0's PV to release slots
                et = epool.tile([P, L], bf16, tag=f"e{kb}", name=f"e{kb}",
                                bufs=4)
                off = kb * P
                pos = 0
                while pos < L:  # 1024-wide psum tiles: 1 exp op per tile
                    c = min(1024, L - pos)
                    ps = pbig.tile([P, 1024], f32, tag="big", name="pssc")
                    for c0 in range(0, c, 512):
                        w = min(512, c - c0)
                        nc.tensor.matmul(
                            ps[:, c0:c0 + w],
                            lhsT=qkT[b][hs, 1, off:off + P],
                            rhs=qkT[b][hs, 0, off + pos + c0:off + pos + c0 + w],
                            start=True, stop=True)
                    nc.scalar.activation(
                        et[:, pos:pos + c], ps[:, :c],
                        mybir.ActivationFunctionType.Exp)
                    pos += c
                # zero the invalid (q < k) half of the diagonal block.
                # DVE (not GpSimd): keeps the gpsimd queue empty so the
                # collective triggers fire as soon as their DMAs land.
                nc.vector.tensor_mul(et[:, 0:P], et[:, 0:P], tri[:])
                expst[kb] = et
                yield

        def pv_pieces(b, e0, e1):
            """Flipped PV for batch b: one (q-tile, head) chain per piece.

            out[d, q] = sum_kb vsb[kb]^T @ expst[kb][:, qwin]: N=512 moving
            columns per matmul, stationary operand only 65 columns, so the
            PE stays matmul-bound (no LDWEIGHTS stalls, no HAM cooldown).
            Row 64 accumulates the softmax denominator (ones column of vsb).
            After both heads' chains for a q-tile: reciprocal rows ->
            K=2 broadcast matmul -> two DVE mults write attnT normalized.
            """
            expst = (e0, e1)
            for qt in range(NQT):
                q0 = qt * QT
                pvs = [None, None]
                for h in range(HPC):
                    pp = ppv.tile([65, QT], f32, tag=f"pv{h}",
                                  name=f"pv{h}")
                    nkb = 4 * qt + 4  # k-blocks touching this q-tile
                    for kb in range(nkb):
                        ecol = q0 - kb * P  # expst col of q-tile start
                        poff = max(0, -ecol)
                        w = QT - poff
                        nc.tensor.matmul(
                            pp[:, poff:QT],
                            lhsT=vsb[b][:, kb, h, 0:65],
                            rhs=expst[h][kb][:, ecol + poff:ecol + poff + w],
                            start=(kb == 0), stop=(kb == nkb - 1))
                    # fast-release: one DVE copy frees the PSUM slot so the
                    # next chain never waits on the normalize tail
                    pvs[h] = small.tile([65, QT], bf16, tag=f"pvs{h}",
                                        name=f"pvs{h}", bufs=1)
                    nc.vector.tensor_copy(pvs[h][:], pp[:, :])
                    yield
                # broadcast each raw denominator row across 64 partitions
                # with a K=1 matmul, take the reciprocal on the broadcast
                # (per-lane cost is free-dim-bound, so this costs the same
                # as a single-row reciprocal but needs no extra copy), then
                # normalize into the head's attnT half
                for h in range(HPC):
                    bc = psm.tile([D, QT], f32, tag="mid", name="bc")
                    nc.tensor.matmul(bc[0:D, :],
                                     lhsT=onesc[64:65, 0:D],
                                     rhs=pvs[h][64:65, :],
                                     start=True, stop=True)
                    bcs = small.tile([D, QT], f32, tag="bcs",
                                     name=f"bcs{h}", bufs=1)
                    nc.vector.reciprocal_approx_fast(out=bcs[:],
                                                     in_=bc[0:D, :])
                    nc.vector.tensor_mul(attnT[b][h][0:D, q0:q0 + QT],
                                         pvs[h][0:D, :], bcs[0:D, :])
                yield

        def interleave(*gens):
            gens = list(gens)
            while gens:
                gens = [g for g in gens if next(g, StopIteration) is not StopIteration]

        def paced(qg, score_gens, pv_gens=(), pv_every=1):
            """Weave one qkv stream with score/pv streams, pacing emission so
            every score k-block is emitted AFTER the qkv chunk that writes the
            qkT columns it reads (Tile only tracks writer->reader deps in
            emission order). qkv chunk g (suffix-first) unlocks score k-blocks
            [12-4g, 15-4g]."""
            rnd = 0
            for g in range(4):
                for _ in range(6):
                    next(qg, None)
                for _ in range(4):
                    for sg in score_gens:
                        next(sg, None)
                    if rnd % pv_every == 0:
                        for pg in pv_gens:
                            next(pg, None)
                    rnd += 1
            interleave(qg, *score_gens, *pv_gens)

        atf = [work.tile([P, EB, 2 * P], bf16, tag="atf",
                         name=f"atf{b}") for b in range(B)]

        def bounce(b):
            """attnT -> a2a_in: chunk j of the bounce gets token blocks
            {j, j+8}; head h's 64 rows land at payload partitions h*64+.
            On the otherwise-empty GpSimd SWDGE queue so the collective
            trigger right behind it fires immediately."""
            for t in range(2):
                for h in range(HPC):
                    nc.gpsimd.dma_start(
                        a2a_in[b].ap().rearrange(
                            "j p (t c) -> p j t c",
                            t=2)[h * D:(h + 1) * D, :, t, :],
                        attnT[b][h][:, t * NCORES * P:(t + 1) * NCORES * P]
                        .rearrange("p (j c) -> p j c", c=P))

        def a2a_batch(b):
            """AllToAll of batch b (512KB per rank)."""
            if no_cc:
                for j in range(NCORES):
                    nc.sync.dma_start(a2a_out[b][j], a2a_in[b][j])
            else:
                nc.gpsimd.collective_compute(
                    "AllToAll", mybir.AluOpType.bypass,
                    replica_groups=[list(range(NCORES))],
                    ins=[a2a_in[b].ap()], outs=[a2a_out[b].ap()])

        def atf_gather(b):
            nc.sync.dma_start(
                atf[b][:, :, :],
                a2a_out[b].ap().rearrange("j p c -> p j c"))

        def oproj_half(b, st):
            """Output projection of token block st*8 + rank of batch b.
            b_o is host-broadcast to all partitions, so the bias rides the
            PSUM->SBUF copyout as a DVE add (no K=1 bias matmuls)."""
            ot = opool.tile([P, E], bf16, tag="o", name="ot")
            po = pbig.tile([P, 1024], f32, tag="big", name="pso")
            for oh in range(2):
                for eb in range(EB):
                    nc.tensor.matmul(
                        po[:, oh * 512:(oh + 1) * 512],
                        lhsT=atf[b][:, eb, st * P:(st + 1) * P],
                        rhs=woh[oh][:, eb, :],
                        start=(eb == 0), stop=(eb == EB - 1))
            nc.vector.tensor_tensor(out=ot[:], in0=po[:], in1=bo[:],
                                    op=mybir.AluOpType.add)
            nc.sync.dma_start(out_d[b * 2 + st], ot[:])

        # ---- pipelined emission (priorities; Tile schedules by readiness) ----
        # Phase-sequential PE stream (in-order engine queues make fine
        # interleaving counterproductive): batch-0 QKV+scores, batch-1
        # QKV+scores (ACT exps trail), then both PV phases back to back --
        # each triggers its half-AllToAlls as attnT halves complete -- and
        # the four output projections last, overlapping the tail collectives.
        e00, e01, e10, e11 = [], [], [], []
        paced(qkv_pieces(0),
              [score_pieces(0, 0, e00), score_pieces(0, 1, e01)])
        nc.sync.dma_start(bo[:, :], bo_d[:, :])
        # batch 1 middle-out: high k-blocks pace with the suffix-first qkv
        # chunks; the low half is emitted only after ALL qkv1 pieces (its
        # matmuls read every qkT column -- emission order must respect
        # writer->reader) and runs forward so pv(1) unlocks incrementally.
        # k-blocks 7..4 need only qkv1 chunks sc>=1 (stationary k-cols
        # 512-1023, moving q-cols >= 512), so they pace inside phase 2's
        # rounds; only kb 0-3 must trail the final chunk -- this pulls
        # ~10us of exp off the post-projection critical path
        mid_hi = list(reversed(range(8, SBB))) + [7, 6, 5, 4]
        mid_lo = list(range(4))
        paced(qkv_pieces(1),
              [score_pieces(1, 0, e10, mid_hi),
               score_pieces(1, 1, e11, mid_hi)])
        # W_o halves land in the two xc slots the moment QKV stops using
        # them (same shape/tag); loaded well before the first oproj
        woh = [xpool.tile([P, EB, 512], bf16, tag="xc", name=f"wo{oh}")
               for oh in range(2)]
        for oh in range(2):
            nc.sync.dma_start(woh[oh][:], wo_d[:, :, oh * 512:(oh + 1) * 512])
        # PV(0) woven WITH batch-1's low score blocks: pv0's exps are long
        # done so its chains fill the PE while scores1lo is ACT-bound, and
        # finishing pv0 here fires A2A(0) ~18us earlier so oproj(0) never
        # waits on it at the tail; scores1lo's exp schedule (and so pv1)
        # is unchanged.
        s1lo = [score_pieces(1, 0, e10, mid_lo),
                score_pieces(1, 1, e11, mid_lo)]
        p0 = pv_pieces(0, e00, e01)
        for _ in range(8):
            for sg in s1lo:
                next(sg, None)
            next(p0, None)
            next(p0, None)
        interleave(p0, *s1lo)
        bounce(0)
        a2a_batch(0)            # overlaps batch-1 PV
        interleave(pv_pieces(1, e10, e11))
        bounce(1)
        a2a_batch(1)            # overlaps oproj of batch 0
        atf_gather(0)
        atf_gather(1)
        oproj_half(0, 0)
        oproj_half(0, 1)
        oproj_half(1, 0)
        oproj_half(1, 1)

    nc.compile()
    return nc


def _in_maps(x, W_qkv, b_qkv, W_o, b_o):
    # [partition, eblock, col] layouts (see dram tensor decls)
    xT = np.ascontiguousarray(
        x.reshape(BS, EB, P).transpose(2, 1, 0)).astype(_bf16)
    wo = np.ascontiguousarray(
        W_o.reshape(EB, P, E).transpose(1, 0, 2)).astype(_bf16)
    # fold the v bias through the output projection: softmax rows sum to
    # 1, so attn = softmax@v + bv and out = softmax@v@W_o + (bv@W_o + b_o)
    bo2 = np.asarray(b_o, np.float64) + np.asarray(
        b_qkv[2 * E:], np.float64) @ np.asarray(W_o, np.float64)
    bo = np.ascontiguousarray(np.broadcast_to(
        bo2.reshape(1, E), (P, E))).astype(_bf16)
    tri = np.triu(np.ones((P, P), np.float32)).astype(_bf16)
    maps = []
    for c in range(NCORES):
        o = c * HPC * D
        q_sl = slice(o, o + HPC * D)
        k_sl = slice(E + o, E + o + HPC * D)
        v_sl = slice(2 * E + o, 2 * E + o + HPC * D)
        wqk = np.concatenate(
            [W_qkv[:, q_sl] * 0.125, W_qkv[:, k_sl]], axis=1)
        maps.append({
            "xT": xT,
            "wqk": np.ascontiguousarray(
                wqk.reshape(EB, P, 2 * P).transpose(1, 0, 2)).astype(_bf16),
            "wv": np.ascontiguousarray(
                W_qkv[:, v_sl].reshape(EB, P, P).transpose(1, 0, 2)).astype(_bf16),
            "wo": wo,
            "bqk": np.stack([b_qkv[q_sl] * 0.125,
                             b_qkv[k_sl]], axis=1).astype(np.float32),
            "bo": bo,
            "tri": tri,
        })
    return maps


def kernel(x, W_qkv, b_qkv, W_o, b_o, mask):
    from concourse.bass_utils import run_bass_kernel_spmd

    if "nc" not in _cache:
        _cache["nc"] = _build()
    nc = _cache["nc"]
    maps = _in_maps(np.asarray(x, np.float32), np.asarray(W_qkv, np.float32),
                    np.asarray(b_qkv, np.float32), np.asarray(W_o, np.float32),
                    np.asarray(b_o, np.float32))
    res = run_bass_kernel_spmd(nc, maps, list(range(NCORES)))
    # rank r's out[st] is global 128-token block st*8 + r
    full = np.empty((SB, P, E), np.float32)
    for r in range(NCORES):
        full[r::NCORES] = res.results[r]["out"]
    return full.reshape(B, S, E).astype(np.float32)

